# revision 33
# baseline (speedup 1.0000x reference)
"""TRN2 Bass kernel for nn_MultiHeadAttn_1580547971654.

Multi-head attention with sigmoid activation (no softmax normalization),
2D key-side mask. query [2,1024,1024], key/value [2,2048,1024],
Wq/Wk/Wv [1024,1024], Wo [1024,1024], NH=16, HD=64.

Sharding (8 cores): data-parallel over batch (2) x tensor-parallel over
head groups (4 groups of 4 heads).  Core (b, g) computes
  partial[b] = sigmoid(scale * (q[b] Wq[:,G]) (k[b] Wk[:,G])^T) ((v[b]*mask) Wv[:,G]) Wo[G,:]
with G = head-group g's 256-wide hidden slice.  Host sums 4 partials per
batch.

Mask compaction: masked klen positions contribute exactly zero
(reference: sigmoid(-1e30) == 0), so the host gathers only unmasked
key/value columns, zero-padded to a multiple of 128.  With the uniform
0/1 mask this halves the klen-side work exactly.

Numerics: fp16 operands everywhere (TRN2 PE does native fp16 multiplies
with fp32 PSUM accumulation), so the only error is rounding tensors to
fp16 (2^-11).  Scale is folded into the sigmoid activation's scale.

Layout: activations are uploaded pre-transposed ([hidden, len]) so all
matmuls contract over the partition axis with no on-device transposes.
Per-head score matmuls (K=64) are row-packed in pairs into PE rows 0-63 /
64-127; attn@V matmuls (M=64) are col-packed in pairs.

Schedule: emission order defines both Tile's dependencies and each
engine's runtime execution order, so it is laid out to match expected
runtime readiness.  The sigmoid stream is the spine (one score-pair
lookahead + sigmoid per slot); all other PE work (attn@V pairs,
projections, out-proj) is cut into small atoms in a queue with release
slots derived from a calibrated DMA-arrival model (~0.7us per dma_start
+ ~2.9us/MB, serial) and deadline slots from dataflow need, popped two
per slot between sigmoid groups.  attn@V trails its sigmoid via a deep
psb pool (bufs=20) so late V-block arrival cannot stall ScalarE.
Weights are packed into two DMAs (wq|wk, wv|wo) to cut per-dma_start
issue overhead on the critical first-sigmoid path.  Input tile pool
holds every block (no WAR DMA stalls); the warmup DCE-keeper DMA goes
to a separate dummy output so it cannot block the DMA queue.  The tail
finishes out_proj(1) with per-qt avt casts, PSUM borrowed from the
score pool, evac casts split across Vector/Scalar, and whole
[128,1024]-row output DMAs.
"""

import numpy as np

BSZ, QLEN, KLEN = 2, 1024, 2048
HID = 1024
NH, HD = 16, 64
SCALE = 1.0 / (HD ** 0.5)
N_CORES = 8
GSLICE = 256           # hidden slice per core (4 heads = 2 head-pairs)
P = 128

_cache = {}


def _build(nkt):
    import concourse.bass as bass
    import concourse.tile as tile
    from concourse import bacc, mybir

    f32 = mybir.dt.float32
    f16 = mybir.dt.float16
    SIG = mybir.ActivationFunctionType.Sigmoid

    klen_c = nkt * P          # compacted + padded klen
    blocks = []
    pos = 0
    while pos < klen_c:
        blocks.append((pos, min(512, klen_c - pos)))
        pos += 512
    nblk = len(blocks)

    nc = bacc.Bacc("TRN2", target_bir_lowering=False, debug=False,
                   num_devices=N_CORES)

    # Pre-blocked inputs: x[blk, p, c, l] = x_T[c*128+p, blk*512+l].
    qT_v = nc.dram_tensor("qT", [2, P, HID // P, 512], f16, kind="ExternalInput").ap()
    kT_v = nc.dram_tensor("kT", [nblk, P, HID // P, 512], f16, kind="ExternalInput").ap()
    vT_v = nc.dram_tensor("vT", [nblk, P, HID // P, 512], f16, kind="ExternalInput").ap()
    # weights as four contiguous 0.25MB blocks [wk0|wq0|wq1|wk1] so
    # every weight DMA is a whole contiguous block (strided sources run
    # at ~half the HBM rate)
    wkq_v = nc.dram_tensor("wkq", [4, P, HID // P, P], f16, kind="ExternalInput").ap()
    wvo_v = nc.dram_tensor("wvo", [2, P, 2 * HID], f16, kind="ExternalInput").ap()
    po_ap = nc.dram_tensor("po", [QLEN, HID], f16, kind="ExternalOutput").ap()
    dump_ap = nc.dram_tensor("dump", [1, 1], f16, kind="ExternalOutput").ap()

    NC_ = HID // P      # 8 contraction chunks

    with tile.TileContext(nc) as tc:
        with tc.tile_pool(name="sb", bufs=1) as sb, \
             tc.tile_pool(name="xin", bufs=2 * nblk + 2) as xin_pool, \
             tc.tile_pool(name="pt", bufs=20) as pt_pool, \
             tc.tile_pool(name="ost", bufs=4) as ost_pool, \
             tc.tile_pool(name="mm", bufs=2, space="PSUM") as mm_pool, \
             tc.tile_pool(name="av", bufs=2, space="PSUM") as av_pool, \
             tc.tile_pool(name="sps", bufs=2, space="PSUM") as s_pool:

            # ---- persistent tiles ----
            # [P, half(head-pair), kind(wq,wk), c, 128]
            wkq_sb = sb.tile([P, 2, 2, NC_, P], f16, tag="wkq")
            wvo_sb = sb.tile([P, 2, 2 * HID], f16, tag="wvo")

            v_sb = sb.tile([P, nkt, GSLICE], f16, tag="v")      # V natural [klen_c, 256]
            kt_sb = sb.tile([P, 2, klen_c], f16, tag="kt")      # K^T [hd(2x128), klen_c]
            qt_sb = sb.tile([P, 2, QLEN], f16, tag="qt")        # Q^T [hd, qlen]
            avt_sb = sb.tile([P, 2, 2, 512], f16, tag="avt")    # AV^T [hd, pair, qc, q]

            xq_t, xk_t, xv_t = {}, {}, {}

            # ---- DMA issue (order = priority = arrival urgency) ----
            def dma_x(store, dram, blk, chunks, nm=""):
                x = xin_pool.tile([P, NC_, 512], f16, tag="xin",
                                  name=f"x{nm}{blk}")
                blen = blocks[blk][1] if dram is not qT_v else 512
                for cc in range(0, NC_, chunks):
                    nc.sync.dma_start(out=x[:, cc:cc + chunks, 0:blen],
                                      in_=dram[blk, :, cc:cc + chunks, 0:blen])
                store[blk] = x

            nc.sync.dma_start(out=wkq_sb[:, 0, 1], in_=wkq_v[0])  # wk h0
            dma_x(xk_t, kT_v, 0, 8, "k")         # xk0 (one instr)
            nc.sync.dma_start(out=wkq_sb[:, 0, 0], in_=wkq_v[1])  # wq h0
            dma_x(xq_t, qT_v, 0, 4, "q")         # xq0 c0-3, c4-7
            for blk in range(2, nblk):
                dma_x(xk_t, kT_v, blk, 8, "k")   # xk2 (small tail block)
            nc.sync.dma_start(out=wkq_sb[:, 1, 0], in_=wkq_v[2])  # wq h1
            nc.sync.dma_start(out=wkq_sb[:, 1, 1], in_=wkq_v[3])  # wk h1
            dma_x(xk_t, kT_v, 1, 8, "k")         # xk1
            nc.sync.dma_start(out=wvo_sb[:, 0], in_=wvo_v[0])   # wv
            for blk in range(nblk):
                dma_x(xv_t, vT_v, blk, 8, "v")
            dma_x(xq_t, qT_v, 1, 8, "q")
            nc.sync.dma_start(out=wvo_sb[:, 1], in_=wvo_v[1])   # wo

            # ---- PE warm-up (keeps HAM at 2.4 GHz until real work) ----
            # Warmup matmuls go to an s_pool PSUM tile (not mm_pool) so
            # both mm bufs stay free for the first K/Q projections, and
            # more warmups can be interleaved into pre-spine DMA gaps.
            wtmp = sb.tile([P, 512], f16, tag="wtmp")
            nc.vector.memset(wtmp[:], 0.0)
            warm_ps = s_pool.tile([P, 2, 512], f32, tag="s", name="warm")

            def warm(n):
                for _ in range(n):
                    nc.tensor.matmul(warm_ps[:, 0, :], wtmp[:, 0:128],
                                     wtmp[:], start=True, stop=True)
            warm(13)

            # ---- attention primitives ----
            av_tiles = {}
            avt_done = set()

            def score(qc, pair, kt):
                sps = s_pool.tile([P, 2, 512], f32, tag="s",
                                  name=f"s{qc}_{pair}_{kt}")
                for h in range(2):
                    nc.tensor.matmul(
                        sps[:, h, :],
                        kt_sb[64 * h:64 * h + 64, pair, kt * P:(kt + 1) * P],
                        qt_sb[64 * h:64 * h + 64, pair, qc * 512:(qc + 1) * 512],
                        start=True, stop=True,
                    )
                return sps

            def sig(qc, pair, kt, sps):
                psb = pt_pool.tile([P, 2, 512], f16, tag="p",
                                   name=f"p{qc}_{pair}_{kt}")
                nc.scalar.activation(psb[:], sps[:], SIG, scale=float(SCALE))
                return psb

            def av(qc, pair, kt, psb, first, last):
                if (qc, pair) not in av_tiles:
                    av_tiles[(qc, pair)] = av_pool.tile(
                        [P, 512], f32, tag="av", name=f"av_{qc}_{pair}")
                avps = av_tiles[(qc, pair)]
                for h in range(2):
                    nc.tensor.matmul(
                        avps[64 * h:64 * h + 64, :],
                        v_sb[:, kt, pair * P + 64 * h: pair * P + 64 * h + 64],
                        psb[:, h, :],
                        start=first, stop=last,
                    )
                if last:
                    if (qc, pair) == (1, 1):
                        # final pass: per-qt casts so the tail out_proj
                        # matmuls start as soon as their column lands
                        for qt in range(4):
                            nc.vector.tensor_copy(
                                avt_sb[:, pair, qc, qt * P:(qt + 1) * P],
                                avps[:, qt * P:(qt + 1) * P])
                    else:
                        nc.vector.tensor_copy(avt_sb[:, pair, qc, :],
                                              avps[:])
                    del av_tiles[(qc, pair)]
                    avt_done.add((qc, pair))

            def op_nn(qc, qt, nn, ost_box, cast_eng):
                if '' not in ost_box:
                    ost_box[''] = ost_pool.tile([P, 2, 512], f16, tag="ost",
                                                name=f"os{qc}_{qt}")
                ost = ost_box['']
                ops = mm_pool.tile([P, 512], f32, tag="mm",
                                   name=f"o{qc}_{qt}_{nn}")
                for pr in range(2):
                    nc.tensor.matmul(
                        ops[:],
                        avt_sb[:, pr, qc, qt * P:(qt + 1) * P],
                        wvo_sb[:, 1, pr * HID + nn * 512:pr * HID + (nn + 1) * 512],
                        start=(pr == 0), stop=(pr == 1),
                    )
                if cast_eng == 's':
                    nc.scalar.copy(ost[:, nn, :], ops[:])
                else:
                    nc.vector.tensor_copy(ost[:, nn, :], ops[:])
                r0 = qc * 512 + qt * P
                nc.sync.dma_start(
                    out=po_ap[r0:r0 + P, nn * 512:(nn + 1) * 512],
                    in_=ost[:, nn, :])

            # ---- virtual-time emission ----
            # Emission order defines Tile's dependencies and each
            # engine's runtime execution order.  A virtual PE clock is
            # advanced as instructions are emitted; queued atoms pop
            # between score pairs only while the projected PE time stays
            # below the point where the next score pair must start.
            # Deadlines are also the emission-order CORRECTNESS bound:
            # a producer atom must be emitted before the spine op that
            # reads its output (Tile deps follow emission order).
            passes = [(0, 0), (0, 1), (1, 0), (1, 1)]

            def tiles_of(b):
                t0 = blocks[b][0] // P
                return list(range(t0, t0 + blocks[b][1] // P))

            kt_nat = list(range(nkt))
            if nblk >= 3:
                # pass 0 visits the small tail block before block 1: its
                # tiny projection depends on xk2 which is DMA'd before
                # the big xk1, buying time for xk1 to land
                kt_seq0 = tiles_of(0) + tiles_of(nblk - 1)
                for b in range(1, nblk - 1):
                    kt_seq0 += tiles_of(b)
            else:
                kt_seq0 = kt_nat
            kt_seqs = [kt_seq0, kt_nat, kt_nat, kt_nat]
            slots = []
            for p, (qc, pair) in enumerate(passes):
                for si, kt in enumerate(kt_seqs[p]):
                    slots.append((qc, pair, kt, si == 0, si == nkt - 1))
            nslots = len(slots)

            def blk_of(kt):
                for bi in range(nblk):
                    t0 = blocks[bi][0] // P
                    if t0 <= kt < t0 + blocks[bi][1] // P:
                        return bi

            need_k, need_q = {}, {}
            for s_i, (qc, pair, kt, _f, _l) in enumerate(slots):
                need_k.setdefault((blk_of(kt), pair), s_i)
                need_q.setdefault((qc, pair), s_i)

            # estimated arrival (sem-complete) times: wire-serial at
            # ~2.9us/MB + ~0.3us per dma_start (16-engine completion
            # straggler), first data ~9.6us
            t_arr = {}
            _t = [9.4]

            def land(name, size_mb, n_instr=1):
                _t[0] += size_mb * 2.86 + 0.1 * n_instr
                t_arr[name] = _t[0]
            land('wk0', 0.25)
            land('xk0', blocks[0][1] / 512.)
            land('wq0', 0.25)
            land('xq0', 1., 2)
            for blk in range(2, nblk):
                land(f'xk{blk}', blocks[blk][1] / 512.)
            land('wq1', 0.25)
            land('wk1', 0.25)
            if nblk > 1:
                land('xk1', blocks[1][1] / 512.)
            land('wv', 0.5)
            for blk in range(nblk):
                land(f'xv{blk}', blocks[blk][1] / 512.)
            land('xq1', 1.)
            land('wo', 0.5)

            # queue of atoms; mm-pool users carry a group id so at most
            # two PSUM tiles are ever live (bufs=2) -- a third alloc
            # would wait, at runtime, on a cast emitted after it (hang)
            queue = []
            state = {'open': [], 'vdone': set()}
            vt = {'pe': 0.0}

            def _pop(i):
                e = queue.pop(i)
                e['fn']()
                vt['pe'] = max(vt['pe'], e['rel']) + e['cost']
                gid = e.get('gid')
                if gid is not None:
                    if e.get('opens') and not e.get('closes'):
                        state['open'].append(gid)
                    if e.get('closes') and gid in state['open']:
                        state['open'].remove(gid)
                if e.get('vkt') is not None:
                    state['vdone'].add(e['vkt'])

            def _close_one():
                gid = state['open'][0]
                jj = next(j for j, e in enumerate(queue)
                          if e.get('gid') == gid)
                _pop(jj)

            def pump_until(tlimit, g):
                while True:
                    act = None
                    av_blk = False
                    for i, e in enumerate(queue):
                        forced = e['dl'] is not None and e['dl'] <= g
                        ready = e['rel'] <= vt['pe'] + 0.45
                        fits = vt['pe'] + e['cost'] <= tlimit
                        ok = forced or (ready and fits)
                        if e.get('av'):
                            blocked = av_blk
                            av_blk = True   # AV pops are strictly FIFO
                            if blocked:
                                continue
                            if e['vneed'] not in state['vdone']:
                                if forced:
                                    act = ('force_v', e['vneed'])
                                    break
                                continue
                            if ok:
                                act = ('pop', i)
                                break
                            continue
                        if e.get('navt') and not ({(0, 0), (0, 1)}
                                                  <= avt_done):
                            continue
                        if not ok:
                            continue
                        if e.get('opens') and e['gid'] not in state['open'] \
                                and len(state['open']) >= 2:
                            if forced:
                                act = ('close',)
                                break
                            continue
                        act = ('pop', i)
                        break
                    if act is None:
                        return
                    if act[0] == 'pop':
                        _pop(act[1])
                    elif act[0] == 'close':
                        _close_one()
                    else:
                        while len(state['open']) >= 2:
                            _close_one()
                        jj = next(j for j, e in enumerate(queue)
                                  if e.get('vkt') == act[1])
                        _pop(jj)

            # projection atoms -> queue (two ~0.9us pieces per half)
            def k_atoms(blk, half):
                dl = max(0, need_k[(blk, half)] - 2)
                rel = max(t_arr[f'xk{blk}'], t_arr[f'wk{half}'])
                pos, blen = blocks[blk]
                box = {}
                gid = ('k', blk, half)
                for piece in range(2):
                    def ap(piece=piece, blk=blk, half=half, box=box,
                           pos=pos, blen=blen):
                        if piece == 0:
                            box['t'] = mm_pool.tile(
                                [P, 512], f32, tag="mm",
                                name=f"kps{blk}_{half}")
                        kps = box['t']
                        for c in range(4 * piece, 4 * piece + 4):
                            nc.tensor.matmul(
                                kps[:, 0:blen],
                                wkq_sb[:, half, 1, c, :],
                                xk_t[blk][:, c, 0:blen],
                                start=(c == 0), stop=(c == NC_ - 1))
                        if piece == 1:
                            nc.vector.tensor_copy(
                                kt_sb[:, half, pos:pos + blen],
                                kps[:, 0:blen])
                    queue.append(dict(rel=rel, dl=max(0, dl - 1 + piece),
                                      fn=ap, cost=0.9 * blen / 512, gid=gid,
                                      opens=(piece == 0),
                                      closes=(piece == 1)))

            def q_atoms(qblk, half):
                dl = max(0, need_q[(qblk, half)] - 2)
                rel = max(t_arr[f'xq{qblk}'], t_arr[f'wq{half}'])
                box = {}
                gid = ('q', qblk, half)
                for piece in range(2):
                    def ap(piece=piece, qblk=qblk, half=half, box=box):
                        if piece == 0:
                            box['t'] = mm_pool.tile(
                                [P, 512], f32, tag="mm",
                                name=f"qps{qblk}_{half}")
                        qps = box['t']
                        for c in range(4 * piece, 4 * piece + 4):
                            nc.tensor.matmul(
                                qps[:],
                                wkq_sb[:, half, 0, c, :],
                                xq_t[qblk][:, c, :],
                                start=(c == 0), stop=(c == NC_ - 1))
                        if piece == 1:
                            nc.vector.tensor_copy(
                                qt_sb[:, half, qblk * 512:(qblk + 1) * 512],
                                qps[:])
                    queue.append(dict(rel=rel, dl=max(0, dl - 1 + piece),
                                      fn=ap, cost=0.9,
                                      gid=gid, opens=(piece == 0),
                                      closes=(piece == 1)))

            def v_atoms(blk):
                rel = max(t_arr[f'xv{blk}'], t_arr['wv'])
                pos, blen = blocks[blk]
                for j in range(blen // P):
                    kt_g = pos // P + j

                    def aj(blk=blk, j=j, kt_g=kt_g):
                        vps = mm_pool.tile([P, GSLICE], f32, tag="mm",
                                           name=f"vps{blk}_{j}")
                        for c in range(NC_):
                            nc.tensor.matmul(
                                vps[:],
                                xv_t[blk][:, c, j * P:(j + 1) * P],
                                wvo_sb[:, 0, c * GSLICE:(c + 1) * GSLICE],
                                start=(c == 0), stop=(c == NC_ - 1))
                        nc.vector.tensor_copy(v_sb[:, kt_g, :], vps[:])
                    queue.append(dict(rel=rel, dl=None, fn=aj, cost=0.9,
                                      gid=('v', blk, j), opens=True,
                                      closes=True, vkt=kt_g))

            # ---- spine ----
            # block-0 pair-0 K/Q projections gate slot 0 -- run direct,
            # warmup matmuls interleaved into the DMA-wait gaps so the
            # PE never idles long enough for a HAM MID-window rethrottle
            blen0 = blocks[0][1]
            kps0 = mm_pool.tile([P, 512], f32, tag="mm", name="kps0_0")
            for c in range(4):
                nc.tensor.matmul(kps0[:, 0:blen0], wkq_sb[:, 0, 1, c, :],
                                 xk_t[0][:, c, 0:blen0],
                                 start=(c == 0), stop=False)
            warm(2)
            for c in range(4, NC_):
                nc.tensor.matmul(kps0[:, 0:blen0], wkq_sb[:, 0, 1, c, :],
                                 xk_t[0][:, c, 0:blen0],
                                 start=False, stop=(c == NC_ - 1))
            nc.vector.tensor_copy(kt_sb[:, 0, 0:blen0], kps0[:, 0:blen0])
            warm(2)
            qps0 = mm_pool.tile([P, 512], f32, tag="mm", name="qps0_0")
            for c in range(4):
                nc.tensor.matmul(qps0[:], wkq_sb[:, 0, 0, c, :],
                                 xq_t[0][:, c, :],
                                 start=(c == 0), stop=False)
            warm(2)
            for c in range(4, NC_):
                nc.tensor.matmul(qps0[:], wkq_sb[:, 0, 0, c, :],
                                 xq_t[0][:, c, :],
                                 start=False, stop=(c == NC_ - 1))
            nc.vector.tensor_copy(qt_sb[:, 0, 0:512], qps0[:])
            # free the warmup PSUM tile; keep its result live via dump
            wsb = sb.tile([1, 1], f16, tag="wsb")
            nc.vector.tensor_copy(wsb[:], warm_ps[0:1, 0, 0:1])
            nc.sync.dma_start(out=dump_ap[0:1, 0:1], in_=wsb[:])
            # remaining projections as queue atoms, in rough need order
            for blk in range(1, nblk):
                k_atoms(blk, 0)
            k_atoms(0, 1)
            for blk in range(1, nblk):
                k_atoms(blk, 1)
            q_atoms(0, 1)
            q_atoms(1, 0)
            q_atoms(1, 1)
            for blk in range(nblk):
                v_atoms(blk)

            SIGD = 1.05          # sigmoid instruction + issue (us)
            L1, L2 = 0.12, 0.18  # score->sig and sig->bank-free handoff
            sig_end = {}
            vt['pe'] = t_arr['xq0'] + 1.5
            vt['act'] = 0.0
            sps_cur = score(*slots[0][:3])
            vt['pe'] += 0.42
            sc_done = vt['pe']
            for g, (qc, pair, kt, first, last) in enumerate(slots):
                psb = sig(qc, pair, kt, sps_cur)
                st = max(vt['act'], sc_done + L1)
                sig_end[g] = st + SIGD
                vt['act'] = sig_end[g]
                # AV atom: psb ready at sig end; V tile must be emitted
                # first (enforced via vneed); dl bounds the psb pool WAR
                vblk = blk_of(kt)
                vready = max(t_arr[f'xv{vblk}'], t_arr['wv']) + 1.2
                queue.append(dict(
                    rel=max(sig_end[g] + 0.1, vready), dl=g + 12,
                    fn=(lambda qc=qc, pair=pair, kt=kt, psb=psb,
                        first=first, last=last:
                        av(qc, pair, kt, psb, first, last)),
                    cost=0.38, av=True, vneed=kt))
                if (qc, pair) == (0, 1) and last:
                    avt01_t = sig_end[g] + 1.0
                    for qt in range(4):
                        box = {}
                        for nn in range(2):
                            queue.append(dict(
                                rel=avt01_t + qt * 0.4, dl=22 + qt + nn,
                                fn=(lambda qt=qt, nn=nn, box=box:
                                    op_nn(0, qt, nn, box, 'v')),
                                cost=0.5, gid=('o', qt, nn), opens=True,
                                closes=True, navt=True))
                if g + 1 < nslots:
                    bank_free = sig_end[g - 1] + L2 if g >= 1 else 0.0
                    target = max(sig_end[g] - 0.45, bank_free)
                    pump_until(target, g)
                    sps_cur = score(*slots[g + 1][:3])
                    vt['pe'] = max(vt['pe'], bank_free) + 0.42
                    sc_done = vt['pe']

            # ---- drain ----
            pump_until(1e9, nslots + 100)
            ost = ost_pool.tile([P, 2, 512], f16, tag="ost", name="os1_0")
            for nn in range(2):
                ops = mm_pool.tile([P, 512], f32, tag="mm",
                                   name=f"o1_0_{nn}")
                for pr in range(2):
                    nc.tensor.matmul(
                        ops[:], avt_sb[:, pr, 1, 0:P],
                        wvo_sb[:, 1,
                               pr * HID + nn * 512:pr * HID + (nn + 1) * 512],
                        start=(pr == 0), stop=(pr == 1))
                if nn == 0:
                    nc.vector.tensor_copy(ost[:, nn, :], ops[:])
                else:
                    nc.scalar.copy(ost[:, nn, :], ops[:])
                nc.sync.dma_start(
                    out=po_ap[512:512 + P, nn * 512:(nn + 1) * 512],
                    in_=ost[:, nn, :])
            # remaining out_proj(1) tiles: qt3 through mm_pool (freed by
            # op1 pr1 above) so its matmuls overlap qt1/qt2's evacuation
            box3 = {}
            op_nn(1, 3, 0, box3, 'v')
            op_nn(1, 3, 1, box3, 's')
            # qt1/qt2 in s_pool PSUM (free after the last sigmoid),
            # evac casts split across Vector/Scalar
            for qt in range(1, 3):
                osp = s_pool.tile([P, 2, 512], f32, tag="s",
                                  name=f"osp{qt}")
                for nn in range(2):
                    for pr in range(2):
                        nc.tensor.matmul(
                            osp[:, nn, :],
                            avt_sb[:, pr, 1, qt * P:(qt + 1) * P],
                            wvo_sb[:, 1, pr * HID + nn * 512:pr * HID + (nn + 1) * 512],
                            start=(pr == 0), stop=(pr == 1))
                ostq = ost_pool.tile([P, 2, 512], f16, tag="ost",
                                     name=f"os1_{qt}")
                r0 = 512 + qt * P
                nc.vector.tensor_copy(ostq[:, 0, :], osp[:, 0, :])
                nc.sync.dma_start(out=po_ap[r0:r0 + P, 0:512],
                                  in_=ostq[:, 0, :])
                nc.scalar.copy(ostq[:, 1, :], osp[:, 1, :])
                nc.sync.dma_start(out=po_ap[r0:r0 + P, 512:1024],
                                  in_=ostq[:, 1, :])

    nc.compile()
    return nc


def _prep_in_maps(query, key, value, attn_mask, Wq, Wk, Wv, Wo):
    query = np.asarray(query, np.float32)
    key = np.asarray(key, np.float32)
    value = np.asarray(value, np.float32)
    mask = np.asarray(attn_mask)
    Wq = np.asarray(Wq, np.float32)
    Wk = np.asarray(Wk, np.float32)
    Wv = np.asarray(Wv, np.float32)
    Wo = np.asarray(Wo, np.float32)

    # Masked klen positions contribute exactly 0 (reference: sigmoid(-1e30)
    # == 0), so compact each batch to its unmasked positions, zero-padded
    # to a common multiple of 128.
    idxs = [np.nonzero(mask[b] != 0)[0] for b in range(BSZ)]
    klen_eff = max(len(ix) for ix in idxs)
    nkt = max(4, -(-klen_eff // P))
    klen_c = nkt * P

    nblk = (klen_c + 511) // 512
    klen_pad = nblk * 512

    def block_x(xT, width, pad_to):
        # [HID, width] -> [nblocks, 128, 8, 512] contiguous, zero-padded
        full = np.zeros((HID, pad_to), np.float16)
        full[:, :width] = xT
        nb = pad_to // 512
        return np.ascontiguousarray(
            full.reshape(HID // P, P, nb, 512).transpose(2, 1, 0, 3))

    kTc, vTc = [], []
    for b in range(BSZ):
        ix = idxs[b]
        kTc.append(block_x(key[b].T[:, ix].astype(np.float16), len(ix), klen_pad))
        vTc.append(block_x(value[b].T[:, ix].astype(np.float16), len(ix), klen_pad))

    qT0 = {}
    in_maps = []
    for core in range(N_CORES):
        b, g = divmod(core, 4)
        sl = slice(g * GSLICE, (g + 1) * GSLICE)
        if b not in qT0:
            qT0[b] = block_x(query[b].T.astype(np.float16), QLEN, QLEN)
        wq_h = (Wq[:, sl].astype(np.float16).reshape(HID // P, P, GSLICE)
                .transpose(1, 0, 2))
        wk_h = (Wk[:, sl].astype(np.float16).reshape(HID // P, P, GSLICE)
                .transpose(1, 0, 2))
        wv_h = (Wv[:, sl].astype(np.float16).reshape(HID // P, P, GSLICE)
                .transpose(1, 0, 2))
        wo_h = (Wo[sl, :].astype(np.float16).reshape(2, P, HID)
                .transpose(1, 0, 2))
        def wsplit(w_h, half):
            # [P, NC_, 256] -> [P, NC_, 128] for one head-pair half
            return w_h[:, :, half * P:(half + 1) * P]
        in_maps.append({
            "qT": qT0[b],
            "kT": kTc[b],
            "vT": vTc[b],
            "wkq": np.ascontiguousarray(
                np.stack([wsplit(wk_h, 0), wsplit(wq_h, 0),
                          wsplit(wq_h, 1), wsplit(wk_h, 1)])),
            "wvo": np.ascontiguousarray(
                np.stack([wv_h.reshape(P, 2 * HID),
                          wo_h.reshape(P, 2 * HID)])),
        })
    return in_maps, nkt


def _run(in_maps, nkt, trace):
    from concourse.bass_utils import run_bass_kernel_spmd

    if nkt not in _cache:
        _cache[nkt] = _build(nkt)
    res = run_bass_kernel_spmd(_cache[nkt], in_maps, list(range(N_CORES)),
                               trace=trace)
    out = np.zeros((BSZ, QLEN, HID), np.float32)
    for core in range(N_CORES):
        out[core // 4] += res.results[core]["po"].astype(np.float32)
    return out, res


def kernel(query, key, value, attn_mask, Wq, Wk, Wv, Wo):
    in_maps, nkt = _prep_in_maps(query, key, value, attn_mask, Wq, Wk, Wv, Wo)
    out, _ = _run(in_maps, nkt, trace=False)
    return out


def run_traced(query, key, value, attn_mask, Wq, Wk, Wv, Wo):
    """Like kernel() but with NTFF profiling; returns (out, exec_time_ns)."""
    in_maps, nkt = _prep_in_maps(query, key, value, attn_mask, Wq, Wk, Wv, Wo)
    out, res = _run(in_maps, nkt, trace=True)
    return out, res.exec_time_ns



# revision 34
# speedup vs baseline: 1.0412x; 1.0412x over previous
"""TRN2 Bass kernel for nn_MultiHeadAttn_1580547971654.

Multi-head attention with sigmoid activation (no softmax normalization),
2D key-side mask. query [2,1024,1024], key/value [2,2048,1024],
Wq/Wk/Wv [1024,1024], Wo [1024,1024], NH=16, HD=64.

Sharding (8 cores): data-parallel over batch (2) x tensor-parallel over
head groups (4 groups of 4 heads).  Core (b, g) computes
  partial[b] = sigmoid(scale * (q[b] Wq[:,G]) (k[b] Wk[:,G])^T) ((v[b]*mask) Wv[:,G]) Wo[G,:]
with G = head-group g's 256-wide hidden slice.  Host sums 4 partials per
batch.

Mask compaction: masked klen positions contribute exactly zero
(reference: sigmoid(-1e30) == 0), so the host gathers only unmasked
key/value columns, zero-padded to a multiple of 128.  With the uniform
0/1 mask this halves the klen-side work exactly.

Numerics: fp16 operands everywhere (TRN2 PE does native fp16 multiplies
with fp32 PSUM accumulation), so the only error is rounding tensors to
fp16 (2^-11).  Scale is folded into the sigmoid activation's scale.

Layout: activations are uploaded pre-transposed ([hidden, len]) so all
matmuls contract over the partition axis with no on-device transposes.
Per-head score matmuls (K=64) are row-packed in pairs into PE rows 0-63 /
64-127; attn@V matmuls (M=64) are col-packed in pairs.

Schedule: emission order defines both Tile's dependencies and each
engine's runtime execution order, so it is laid out to match expected
runtime readiness.  The sigmoid stream is the spine (one score-pair
lookahead + sigmoid per slot); all other PE work (attn@V pairs,
projections, out-proj) is cut into small atoms in a queue with release
slots derived from a calibrated DMA-arrival model (~0.7us per dma_start
+ ~2.9us/MB, serial) and deadline slots from dataflow need, popped two
per slot between sigmoid groups.  attn@V trails its sigmoid via a deep
psb pool (bufs=20) so late V-block arrival cannot stall ScalarE.
Weights are packed into two DMAs (wq|wk, wv|wo) to cut per-dma_start
issue overhead on the critical first-sigmoid path.  Input tile pool
holds every block (no WAR DMA stalls); the warmup DCE-keeper DMA goes
to a separate dummy output so it cannot block the DMA queue.  The tail
finishes out_proj(1) with per-qt avt casts, PSUM borrowed from the
score pool, evac casts split across Vector/Scalar, and whole
[128,1024]-row output DMAs.
"""

import numpy as np

BSZ, QLEN, KLEN = 2, 1024, 2048
HID = 1024
NH, HD = 16, 64
SCALE = 1.0 / (HD ** 0.5)
N_CORES = 8
GSLICE = 256           # hidden slice per core (4 heads = 2 head-pairs)
P = 128

_cache = {}


def _build(nkt):
    import concourse.bass as bass
    import concourse.tile as tile
    from concourse import bacc, mybir

    f32 = mybir.dt.float32
    f16 = mybir.dt.float16
    SIG = mybir.ActivationFunctionType.Sigmoid

    klen_c = nkt * P          # compacted + padded klen
    blocks = []
    pos = 0
    while pos < klen_c:
        blocks.append((pos, min(512, klen_c - pos)))
        pos += 512
    nblk = len(blocks)

    nc = bacc.Bacc("TRN2", target_bir_lowering=False, debug=False,
                   num_devices=N_CORES)

    # Pre-blocked inputs: x[blk, p, c, l] = x_T[c*128+p, blk*512+l].
    qT_v = nc.dram_tensor("qT", [2, P, HID // P, 512], f16, kind="ExternalInput").ap()
    kT_v = nc.dram_tensor("kT", [nblk, P, HID // P, 512], f16, kind="ExternalInput").ap()
    vT_v = nc.dram_tensor("vT", [nblk, P, HID // P, 512], f16, kind="ExternalInput").ap()
    # weights as four contiguous 0.25MB blocks [wk0|wq0|wq1|wk1] so
    # every weight DMA is a whole contiguous block (strided sources run
    # at ~half the HBM rate)
    wkq_v = nc.dram_tensor("wkq", [4, P, HID // P, P], f16, kind="ExternalInput").ap()
    wvo_v = nc.dram_tensor("wvo", [2, P, 2 * HID], f16, kind="ExternalInput").ap()
    po_ap = nc.dram_tensor("po", [QLEN, HID], f16, kind="ExternalOutput").ap()
    dump_ap = nc.dram_tensor("dump", [1, 1], f16, kind="ExternalOutput").ap()

    NC_ = HID // P      # 8 contraction chunks

    with tile.TileContext(nc) as tc:
        with tc.tile_pool(name="sb", bufs=1) as sb, \
             tc.tile_pool(name="xin", bufs=2 * nblk + 2) as xin_pool, \
             tc.tile_pool(name="pt", bufs=20) as pt_pool, \
             tc.tile_pool(name="ost", bufs=4) as ost_pool, \
             tc.tile_pool(name="mm", bufs=2, space="PSUM") as mm_pool, \
             tc.tile_pool(name="av", bufs=2, space="PSUM") as av_pool, \
             tc.tile_pool(name="sps", bufs=2, space="PSUM") as s_pool:

            # ---- persistent tiles ----
            # [P, half(head-pair), kind(wq,wk), c, 128]
            wkq_sb = sb.tile([P, 2, 2, NC_, P], f16, tag="wkq")
            wvo_sb = sb.tile([P, 2, 2 * HID], f16, tag="wvo")

            v_sb = sb.tile([P, nkt, GSLICE], f16, tag="v")      # V natural [klen_c, 256]
            kt_sb = sb.tile([P, 2, klen_c], f16, tag="kt")      # K^T [hd(2x128), klen_c]
            qt_sb = sb.tile([P, 2, QLEN], f16, tag="qt")        # Q^T [hd, qlen]
            avt_sb = sb.tile([P, 2, 2, 512], f16, tag="avt")    # AV^T [hd, pair, qc, q]

            xq_t, xk_t, xv_t = {}, {}, {}

            # ---- DMA issue (order = priority = arrival urgency) ----
            def dma_x(store, dram, blk, chunks, nm=""):
                x = xin_pool.tile([P, NC_, 512], f16, tag="xin",
                                  name=f"x{nm}{blk}")
                blen = blocks[blk][1] if dram is not qT_v else 512
                for cc in range(0, NC_, chunks):
                    nc.sync.dma_start(out=x[:, cc:cc + chunks, 0:blen],
                                      in_=dram[blk, :, cc:cc + chunks, 0:blen])
                store[blk] = x

            nc.sync.dma_start(out=wkq_sb[:, 0, 1], in_=wkq_v[0])  # wk h0
            dma_x(xk_t, kT_v, 0, 8, "k")         # xk0 (one instr)
            nc.sync.dma_start(out=wkq_sb[:, 0, 0], in_=wkq_v[1])  # wq h0
            dma_x(xq_t, qT_v, 0, 4, "q")         # xq0 c0-3, c4-7
            for blk in range(2, nblk):
                dma_x(xk_t, kT_v, blk, 8, "k")   # xk2 (small tail block)
            nc.sync.dma_start(out=wkq_sb[:, 1, 0], in_=wkq_v[2])  # wq h1
            nc.sync.dma_start(out=wkq_sb[:, 1, 1], in_=wkq_v[3])  # wk h1
            dma_x(xk_t, kT_v, 1, 8, "k")         # xk1
            nc.sync.dma_start(out=wvo_sb[:, 0], in_=wvo_v[0])   # wv
            dma_x(xv_t, vT_v, 0, 8, "v")
            dma_x(xq_t, qT_v, 1, 8, "q")
            for blk in range(1, nblk):
                dma_x(xv_t, vT_v, blk, 8, "v")
            nc.sync.dma_start(out=wvo_sb[:, 1], in_=wvo_v[1])   # wo

            # ---- PE warm-up (keeps HAM at 2.4 GHz until real work) ----
            # Warmup matmuls go to an s_pool PSUM tile (not mm_pool) so
            # both mm bufs stay free for the first K/Q projections, and
            # more warmups can be interleaved into pre-spine DMA gaps.
            wtmp = sb.tile([P, 512], f16, tag="wtmp")
            nc.vector.memset(wtmp[:], 0.0)
            warm_ps = s_pool.tile([P, 2, 512], f32, tag="s", name="warm")

            def warm(n):
                for _ in range(n):
                    nc.tensor.matmul(warm_ps[:, 0, :], wtmp[:, 0:128],
                                     wtmp[:], start=True, stop=True)
            warm(13)

            # ---- attention primitives ----
            av_tiles = {}
            avt_done = set()

            def score(qc, pair, kt):
                sps = s_pool.tile([P, 2, 512], f32, tag="s",
                                  name=f"s{qc}_{pair}_{kt}")
                for h in range(2):
                    nc.tensor.matmul(
                        sps[:, h, :],
                        kt_sb[64 * h:64 * h + 64, pair, kt * P:(kt + 1) * P],
                        qt_sb[64 * h:64 * h + 64, pair, qc * 512:(qc + 1) * 512],
                        start=True, stop=True,
                    )
                return sps

            def sig(qc, pair, kt, sps):
                psb = pt_pool.tile([P, 2, 512], f16, tag="p",
                                   name=f"p{qc}_{pair}_{kt}")
                nc.scalar.activation(psb[:], sps[:], SIG, scale=float(SCALE))
                return psb

            def av(qc, pair, kt, psb, first, last):
                if (qc, pair) not in av_tiles:
                    av_tiles[(qc, pair)] = av_pool.tile(
                        [P, 512], f32, tag="av", name=f"av_{qc}_{pair}")
                avps = av_tiles[(qc, pair)]
                for h in range(2):
                    nc.tensor.matmul(
                        avps[64 * h:64 * h + 64, :],
                        v_sb[:, kt, pair * P + 64 * h: pair * P + 64 * h + 64],
                        psb[:, h, :],
                        start=first, stop=last,
                    )
                if last:
                    if (qc, pair) == (1, 1):
                        # final pass: per-qt casts so the tail out_proj
                        # matmuls start as soon as their column lands
                        for qt in range(4):
                            nc.vector.tensor_copy(
                                avt_sb[:, pair, qc, qt * P:(qt + 1) * P],
                                avps[:, qt * P:(qt + 1) * P])
                    else:
                        nc.vector.tensor_copy(avt_sb[:, pair, qc, :],
                                              avps[:])
                    del av_tiles[(qc, pair)]
                    avt_done.add((qc, pair))

            def op_nn(qc, qt, nn, ost_box, cast_eng):
                if '' not in ost_box:
                    ost_box[''] = ost_pool.tile([P, 2, 512], f16, tag="ost",
                                                name=f"os{qc}_{qt}")
                ost = ost_box['']
                ops = mm_pool.tile([P, 512], f32, tag="mm",
                                   name=f"o{qc}_{qt}_{nn}")
                for pr in range(2):
                    nc.tensor.matmul(
                        ops[:],
                        avt_sb[:, pr, qc, qt * P:(qt + 1) * P],
                        wvo_sb[:, 1, pr * HID + nn * 512:pr * HID + (nn + 1) * 512],
                        start=(pr == 0), stop=(pr == 1),
                    )
                if cast_eng == 's':
                    nc.scalar.copy(ost[:, nn, :], ops[:])
                else:
                    nc.vector.tensor_copy(ost[:, nn, :], ops[:])
                r0 = qc * 512 + qt * P
                nc.sync.dma_start(
                    out=po_ap[r0:r0 + P, nn * 512:(nn + 1) * 512],
                    in_=ost[:, nn, :])

            # ---- virtual-time emission ----
            # Emission order defines Tile's dependencies and each
            # engine's runtime execution order.  A virtual PE clock is
            # advanced as instructions are emitted; queued atoms pop
            # between score pairs only while the projected PE time stays
            # below the point where the next score pair must start.
            # Deadlines are also the emission-order CORRECTNESS bound:
            # a producer atom must be emitted before the spine op that
            # reads its output (Tile deps follow emission order).
            passes = [(0, 0), (0, 1), (1, 0), (1, 1)]

            def tiles_of(b):
                t0 = blocks[b][0] // P
                return list(range(t0, t0 + blocks[b][1] // P))

            kt_nat = list(range(nkt))
            if nblk >= 3:
                # pass 0 visits the small tail block before block 1: its
                # tiny projection depends on xk2 which is DMA'd before
                # the big xk1, buying time for xk1 to land
                kt_seq0 = tiles_of(0) + tiles_of(nblk - 1)
                for b in range(1, nblk - 1):
                    kt_seq0 += tiles_of(b)
            else:
                kt_seq0 = kt_nat
            kt_seqs = [kt_seq0, kt_nat, kt_nat, kt_nat]
            slots = []
            for p, (qc, pair) in enumerate(passes):
                for si, kt in enumerate(kt_seqs[p]):
                    slots.append((qc, pair, kt, si == 0, si == nkt - 1))
            nslots = len(slots)

            def blk_of(kt):
                for bi in range(nblk):
                    t0 = blocks[bi][0] // P
                    if t0 <= kt < t0 + blocks[bi][1] // P:
                        return bi

            need_k, need_q = {}, {}
            for s_i, (qc, pair, kt, _f, _l) in enumerate(slots):
                need_k.setdefault((blk_of(kt), pair), s_i)
                need_q.setdefault((qc, pair), s_i)

            # estimated arrival (sem-complete) times: wire-serial at
            # ~2.9us/MB + ~0.3us per dma_start (16-engine completion
            # straggler), first data ~9.6us
            t_arr = {}
            _t = [9.4]

            def land(name, size_mb, n_instr=1):
                _t[0] += size_mb * 2.86 + 0.1 * n_instr
                t_arr[name] = _t[0]
            land('wk0', 0.25)
            land('xk0', blocks[0][1] / 512.)
            land('wq0', 0.25)
            land('xq0', 1., 2)
            for blk in range(2, nblk):
                land(f'xk{blk}', blocks[blk][1] / 512.)
            land('wq1', 0.25)
            land('wk1', 0.25)
            if nblk > 1:
                land('xk1', blocks[1][1] / 512.)
            land('wv', 0.5)
            land('xv0', blocks[0][1] / 512.)
            land('xq1', 1.)
            for blk in range(1, nblk):
                land(f'xv{blk}', blocks[blk][1] / 512.)
            land('wo', 0.5)

            # queue of atoms; mm-pool users carry a group id so at most
            # two PSUM tiles are ever live (bufs=2) -- a third alloc
            # would wait, at runtime, on a cast emitted after it (hang)
            queue = []
            state = {'open': [], 'vdone': set()}
            vt = {'pe': 0.0}

            def _pop(i):
                e = queue.pop(i)
                e['fn']()
                vt['pe'] = max(vt['pe'], e['rel']) + e['cost']
                gid = e.get('gid')
                if gid is not None:
                    if e.get('opens') and not e.get('closes'):
                        state['open'].append(gid)
                    if e.get('closes') and gid in state['open']:
                        state['open'].remove(gid)
                if e.get('vkt') is not None:
                    state['vdone'].add(e['vkt'])

            def _close_one():
                gid = state['open'][0]
                jj = next(j for j, e in enumerate(queue)
                          if e.get('gid') == gid)
                _pop(jj)

            def pump_until(tlimit, g):
                while True:
                    act = None
                    av_blk = False
                    for i, e in enumerate(queue):
                        forced = e['dl'] is not None and e['dl'] <= g
                        ready = e['rel'] <= vt['pe'] + 0.45
                        fits = vt['pe'] + e['cost'] <= tlimit
                        ok = forced or (ready and fits)
                        if e.get('av'):
                            blocked = av_blk
                            av_blk = True   # AV pops are strictly FIFO
                            if blocked:
                                continue
                            if e['vneed'] not in state['vdone']:
                                if forced:
                                    act = ('force_v', e['vneed'])
                                    break
                                continue
                            if ok:
                                act = ('pop', i)
                                break
                            continue
                        if e.get('navt') and not ({(0, 0), (0, 1)}
                                                  <= avt_done):
                            continue
                        if not ok:
                            continue
                        if e.get('opens') and e['gid'] not in state['open'] \
                                and len(state['open']) >= 2:
                            if forced:
                                act = ('close',)
                                break
                            continue
                        act = ('pop', i)
                        break
                    if act is None:
                        return
                    if act[0] == 'pop':
                        _pop(act[1])
                    elif act[0] == 'close':
                        _close_one()
                    else:
                        while len(state['open']) >= 2:
                            _close_one()
                        jj = next(j for j, e in enumerate(queue)
                                  if e.get('vkt') == act[1])
                        _pop(jj)

            # projection atoms -> queue (two ~0.9us pieces per half)
            def k_atoms(blk, half):
                dl = max(0, need_k[(blk, half)] - 2)
                rel = max(t_arr[f'xk{blk}'], t_arr[f'wk{half}'])
                pos, blen = blocks[blk]
                box = {}
                gid = ('k', blk, half)
                for piece in range(2):
                    def ap(piece=piece, blk=blk, half=half, box=box,
                           pos=pos, blen=blen):
                        if piece == 0:
                            box['t'] = mm_pool.tile(
                                [P, 512], f32, tag="mm",
                                name=f"kps{blk}_{half}")
                        kps = box['t']
                        for c in range(4 * piece, 4 * piece + 4):
                            nc.tensor.matmul(
                                kps[:, 0:blen],
                                wkq_sb[:, half, 1, c, :],
                                xk_t[blk][:, c, 0:blen],
                                start=(c == 0), stop=(c == NC_ - 1))
                        if piece == 1:
                            nc.vector.tensor_copy(
                                kt_sb[:, half, pos:pos + blen],
                                kps[:, 0:blen])
                    queue.append(dict(rel=rel, dl=dl, fn=ap,
                                      cost=0.9 * blen / 512, gid=gid,
                                      opens=(piece == 0),
                                      closes=(piece == 1)))

            def q_atoms(qblk, half):
                dl = max(0, need_q[(qblk, half)] - 2)
                rel = max(t_arr[f'xq{qblk}'], t_arr[f'wq{half}'])
                box = {}
                gid = ('q', qblk, half)
                for piece in range(2):
                    def ap(piece=piece, qblk=qblk, half=half, box=box):
                        if piece == 0:
                            box['t'] = mm_pool.tile(
                                [P, 512], f32, tag="mm",
                                name=f"qps{qblk}_{half}")
                        qps = box['t']
                        for c in range(4 * piece, 4 * piece + 4):
                            nc.tensor.matmul(
                                qps[:],
                                wkq_sb[:, half, 0, c, :],
                                xq_t[qblk][:, c, :],
                                start=(c == 0), stop=(c == NC_ - 1))
                        if piece == 1:
                            nc.vector.tensor_copy(
                                qt_sb[:, half, qblk * 512:(qblk + 1) * 512],
                                qps[:])
                    queue.append(dict(rel=rel, dl=dl, fn=ap, cost=0.9,
                                      gid=gid, opens=(piece == 0),
                                      closes=(piece == 1)))

            def v_atoms(blk):
                rel = max(t_arr[f'xv{blk}'], t_arr['wv'])
                pos, blen = blocks[blk]
                for j in range(blen // P):
                    kt_g = pos // P + j

                    def aj(blk=blk, j=j, kt_g=kt_g):
                        vps = mm_pool.tile([P, GSLICE], f32, tag="mm",
                                           name=f"vps{blk}_{j}")
                        for c in range(NC_):
                            nc.tensor.matmul(
                                vps[:],
                                xv_t[blk][:, c, j * P:(j + 1) * P],
                                wvo_sb[:, 0, c * GSLICE:(c + 1) * GSLICE],
                                start=(c == 0), stop=(c == NC_ - 1))
                        nc.vector.tensor_copy(v_sb[:, kt_g, :], vps[:])
                    queue.append(dict(rel=rel, dl=None, fn=aj, cost=0.9,
                                      gid=('v', blk, j), opens=True,
                                      closes=True, vkt=kt_g))

            # ---- spine ----
            # block-0 pair-0 K/Q projections gate slot 0 -- run direct,
            # warmup matmuls interleaved into the DMA-wait gaps so the
            # PE never idles long enough for a HAM MID-window rethrottle
            blen0 = blocks[0][1]
            kps0 = mm_pool.tile([P, 512], f32, tag="mm", name="kps0_0")
            for c in range(4):
                nc.tensor.matmul(kps0[:, 0:blen0], wkq_sb[:, 0, 1, c, :],
                                 xk_t[0][:, c, 0:blen0],
                                 start=(c == 0), stop=False)
            warm(2)
            for c in range(4, NC_):
                nc.tensor.matmul(kps0[:, 0:blen0], wkq_sb[:, 0, 1, c, :],
                                 xk_t[0][:, c, 0:blen0],
                                 start=False, stop=(c == NC_ - 1))
            nc.vector.tensor_copy(kt_sb[:, 0, 0:blen0], kps0[:, 0:blen0])
            warm(2)
            qps0 = mm_pool.tile([P, 512], f32, tag="mm", name="qps0_0")
            for c in range(4):
                nc.tensor.matmul(qps0[:], wkq_sb[:, 0, 0, c, :],
                                 xq_t[0][:, c, :],
                                 start=(c == 0), stop=False)
            warm(2)
            for c in range(4, NC_):
                nc.tensor.matmul(qps0[:], wkq_sb[:, 0, 0, c, :],
                                 xq_t[0][:, c, :],
                                 start=False, stop=(c == NC_ - 1))
            nc.vector.tensor_copy(qt_sb[:, 0, 0:512], qps0[:])
            # free the warmup PSUM tile; keep its result live via dump
            wsb = sb.tile([1, 1], f16, tag="wsb")
            nc.vector.tensor_copy(wsb[:], warm_ps[0:1, 0, 0:1])
            nc.sync.dma_start(out=dump_ap[0:1, 0:1], in_=wsb[:])
            # remaining projections as queue atoms, in rough need order
            for blk in range(1, nblk):
                k_atoms(blk, 0)
            k_atoms(0, 1)
            for blk in range(1, nblk):
                k_atoms(blk, 1)
            q_atoms(0, 1)
            q_atoms(1, 0)
            q_atoms(1, 1)
            for blk in range(nblk):
                v_atoms(blk)

            SIGD = 1.05          # sigmoid instruction + issue (us)
            L1, L2 = 0.12, 0.18  # score->sig and sig->bank-free handoff
            sig_end = {}
            vt['pe'] = t_arr['xq0'] + 1.5
            vt['act'] = 0.0
            sps_cur = score(*slots[0][:3])
            vt['pe'] += 0.42
            sc_done = vt['pe']
            for g, (qc, pair, kt, first, last) in enumerate(slots):
                psb = sig(qc, pair, kt, sps_cur)
                st = max(vt['act'], sc_done + L1)
                sig_end[g] = st + SIGD
                vt['act'] = sig_end[g]
                # AV atom: psb ready at sig end; V tile must be emitted
                # first (enforced via vneed); dl bounds the psb pool WAR
                vblk = blk_of(kt)
                vready = max(t_arr[f'xv{vblk}'], t_arr['wv']) + 1.2
                queue.append(dict(
                    rel=max(sig_end[g] + 0.1, vready), dl=g + 14,
                    fn=(lambda qc=qc, pair=pair, kt=kt, psb=psb,
                        first=first, last=last:
                        av(qc, pair, kt, psb, first, last)),
                    cost=0.38, av=True, vneed=kt))
                if (qc, pair) == (0, 1) and last:
                    avt01_t = sig_end[g] + 1.0
                    for qt in range(4):
                        box = {}
                        for nn in range(2):
                            queue.append(dict(
                                rel=avt01_t + qt * 0.4, dl=nslots - 8 + qt,
                                fn=(lambda qt=qt, nn=nn, box=box:
                                    op_nn(0, qt, nn, box, 'v')),
                                cost=0.5, gid=('o', qt, nn), opens=True,
                                closes=True, navt=True))
                if g + 1 < nslots:
                    bank_free = sig_end[g - 1] + L2 if g >= 1 else 0.0
                    target = max(sig_end[g] - 0.45, bank_free)
                    pump_until(target, g)
                    sps_cur = score(*slots[g + 1][:3])
                    vt['pe'] = max(vt['pe'], bank_free) + 0.42
                    sc_done = vt['pe']

            # ---- drain ----
            pump_until(1e9, nslots + 100)
            ost = ost_pool.tile([P, 2, 512], f16, tag="ost", name="os1_0")
            for nn in range(2):
                ops = mm_pool.tile([P, 512], f32, tag="mm",
                                   name=f"o1_0_{nn}")
                for pr in range(2):
                    nc.tensor.matmul(
                        ops[:], avt_sb[:, pr, 1, 0:P],
                        wvo_sb[:, 1,
                               pr * HID + nn * 512:pr * HID + (nn + 1) * 512],
                        start=(pr == 0), stop=(pr == 1))
                if nn == 0:
                    nc.vector.tensor_copy(ost[:, nn, :], ops[:])
                else:
                    nc.scalar.copy(ost[:, nn, :], ops[:])
                nc.sync.dma_start(
                    out=po_ap[512:512 + P, nn * 512:(nn + 1) * 512],
                    in_=ost[:, nn, :])
            # remaining out_proj(1) tiles: qt3 through mm_pool (freed by
            # op1 pr1 above) so its matmuls overlap qt1/qt2's evacuation
            box3 = {}
            op_nn(1, 3, 0, box3, 'v')
            op_nn(1, 3, 1, box3, 's')
            # qt1/qt2 in s_pool PSUM (free after the last sigmoid),
            # evac casts split across Vector/Scalar
            for qt in range(1, 3):
                osp = s_pool.tile([P, 2, 512], f32, tag="s",
                                  name=f"osp{qt}")
                for nn in range(2):
                    for pr in range(2):
                        nc.tensor.matmul(
                            osp[:, nn, :],
                            avt_sb[:, pr, 1, qt * P:(qt + 1) * P],
                            wvo_sb[:, 1, pr * HID + nn * 512:pr * HID + (nn + 1) * 512],
                            start=(pr == 0), stop=(pr == 1))
                ostq = ost_pool.tile([P, 2, 512], f16, tag="ost",
                                     name=f"os1_{qt}")
                r0 = 512 + qt * P
                nc.vector.tensor_copy(ostq[:, 0, :], osp[:, 0, :])
                nc.sync.dma_start(out=po_ap[r0:r0 + P, 0:512],
                                  in_=ostq[:, 0, :])
                nc.scalar.copy(ostq[:, 1, :], osp[:, 1, :])
                nc.sync.dma_start(out=po_ap[r0:r0 + P, 512:1024],
                                  in_=ostq[:, 1, :])

    nc.compile()
    return nc


def _prep_in_maps(query, key, value, attn_mask, Wq, Wk, Wv, Wo):
    query = np.asarray(query, np.float32)
    key = np.asarray(key, np.float32)
    value = np.asarray(value, np.float32)
    mask = np.asarray(attn_mask)
    Wq = np.asarray(Wq, np.float32)
    Wk = np.asarray(Wk, np.float32)
    Wv = np.asarray(Wv, np.float32)
    Wo = np.asarray(Wo, np.float32)

    # Masked klen positions contribute exactly 0 (reference: sigmoid(-1e30)
    # == 0), so compact each batch to its unmasked positions, zero-padded
    # to a common multiple of 128.
    idxs = [np.nonzero(mask[b] != 0)[0] for b in range(BSZ)]
    klen_eff = max(len(ix) for ix in idxs)
    nkt = max(4, -(-klen_eff // P))
    klen_c = nkt * P

    nblk = (klen_c + 511) // 512
    klen_pad = nblk * 512

    def block_x(xT, width, pad_to):
        # [HID, width] -> [nblocks, 128, 8, 512] contiguous, zero-padded
        full = np.zeros((HID, pad_to), np.float16)
        full[:, :width] = xT
        nb = pad_to // 512
        return np.ascontiguousarray(
            full.reshape(HID // P, P, nb, 512).transpose(2, 1, 0, 3))

    kTc, vTc = [], []
    for b in range(BSZ):
        ix = idxs[b]
        kTc.append(block_x(key[b].T[:, ix].astype(np.float16), len(ix), klen_pad))
        vTc.append(block_x(value[b].T[:, ix].astype(np.float16), len(ix), klen_pad))

    qT0 = {}
    in_maps = []
    for core in range(N_CORES):
        b, g = divmod(core, 4)
        sl = slice(g * GSLICE, (g + 1) * GSLICE)
        if b not in qT0:
            qT0[b] = block_x(query[b].T.astype(np.float16), QLEN, QLEN)
        wq_h = (Wq[:, sl].astype(np.float16).reshape(HID // P, P, GSLICE)
                .transpose(1, 0, 2))
        wk_h = (Wk[:, sl].astype(np.float16).reshape(HID // P, P, GSLICE)
                .transpose(1, 0, 2))
        wv_h = (Wv[:, sl].astype(np.float16).reshape(HID // P, P, GSLICE)
                .transpose(1, 0, 2))
        wo_h = (Wo[sl, :].astype(np.float16).reshape(2, P, HID)
                .transpose(1, 0, 2))
        def wsplit(w_h, half):
            # [P, NC_, 256] -> [P, NC_, 128] for one head-pair half
            return w_h[:, :, half * P:(half + 1) * P]
        in_maps.append({
            "qT": qT0[b],
            "kT": kTc[b],
            "vT": vTc[b],
            "wkq": np.ascontiguousarray(
                np.stack([wsplit(wk_h, 0), wsplit(wq_h, 0),
                          wsplit(wq_h, 1), wsplit(wk_h, 1)])),
            "wvo": np.ascontiguousarray(
                np.stack([wv_h.reshape(P, 2 * HID),
                          wo_h.reshape(P, 2 * HID)])),
        })
    return in_maps, nkt


def _run(in_maps, nkt, trace):
    from concourse.bass_utils import run_bass_kernel_spmd

    if nkt not in _cache:
        _cache[nkt] = _build(nkt)
    res = run_bass_kernel_spmd(_cache[nkt], in_maps, list(range(N_CORES)),
                               trace=trace)
    out = np.zeros((BSZ, QLEN, HID), np.float32)
    for core in range(N_CORES):
        out[core // 4] += res.results[core]["po"].astype(np.float32)
    return out, res


def kernel(query, key, value, attn_mask, Wq, Wk, Wv, Wo):
    in_maps, nkt = _prep_in_maps(query, key, value, attn_mask, Wq, Wk, Wv, Wo)
    out, _ = _run(in_maps, nkt, trace=False)
    return out


def run_traced(query, key, value, attn_mask, Wq, Wk, Wv, Wo):
    """Like kernel() but with NTFF profiling; returns (out, exec_time_ns)."""
    in_maps, nkt = _prep_in_maps(query, key, value, attn_mask, Wq, Wk, Wv, Wo)
    out, res = _run(in_maps, nkt, trace=True)
    return out, res.exec_time_ns



# revision 35
# speedup vs baseline: 1.0480x; 1.0066x over previous
"""TRN2 Bass kernel for nn_MultiHeadAttn_1580547971654.

Multi-head attention with sigmoid activation (no softmax normalization),
2D key-side mask. query [2,1024,1024], key/value [2,2048,1024],
Wq/Wk/Wv [1024,1024], Wo [1024,1024], NH=16, HD=64.

Sharding (8 cores): data-parallel over batch (2) x tensor-parallel over
head groups (4 groups of 4 heads).  Core (b, g) computes
  partial[b] = sigmoid(scale * (q[b] Wq[:,G]) (k[b] Wk[:,G])^T) ((v[b]*mask) Wv[:,G]) Wo[G,:]
with G = head-group g's 256-wide hidden slice.  Host sums 4 partials per
batch.

Mask compaction: masked klen positions contribute exactly zero
(reference: sigmoid(-1e30) == 0), so the host gathers only unmasked
key/value columns, zero-padded to a multiple of 128.  With the uniform
0/1 mask this halves the klen-side work exactly.

Numerics: fp16 operands everywhere (TRN2 PE does native fp16 multiplies
with fp32 PSUM accumulation), so the only error is rounding tensors to
fp16 (2^-11).  Scale is folded into the sigmoid activation's scale.

Layout: activations are uploaded pre-transposed ([hidden, len]) so all
matmuls contract over the partition axis with no on-device transposes.
Per-head score matmuls (K=64) are row-packed in pairs into PE rows 0-63 /
64-127; attn@V matmuls (M=64) are col-packed in pairs.

Schedule: emission order defines both Tile's dependencies and each
engine's runtime execution order, so it is laid out to match expected
runtime readiness.  The sigmoid stream is the spine (one score-pair
lookahead + sigmoid per slot); all other PE work (attn@V pairs,
projections, out-proj) is cut into small atoms in a queue with release
slots derived from a calibrated DMA-arrival model (~0.7us per dma_start
+ ~2.9us/MB, serial) and deadline slots from dataflow need, popped two
per slot between sigmoid groups.  attn@V trails its sigmoid via a deep
psb pool (bufs=20) so late V-block arrival cannot stall ScalarE.
Weights are packed into two DMAs (wq|wk, wv|wo) to cut per-dma_start
issue overhead on the critical first-sigmoid path.  Input tile pool
holds every block (no WAR DMA stalls); the warmup DCE-keeper DMA goes
to a separate dummy output so it cannot block the DMA queue.  The tail
finishes out_proj(1) with per-qt avt casts, PSUM borrowed from the
score pool, evac casts split across Vector/Scalar, and whole
[128,1024]-row output DMAs.
"""

import numpy as np

BSZ, QLEN, KLEN = 2, 1024, 2048
HID = 1024
NH, HD = 16, 64
SCALE = 1.0 / (HD ** 0.5)
N_CORES = 8
GSLICE = 256           # hidden slice per core (4 heads = 2 head-pairs)
P = 128

_cache = {}


def _build(nkt):
    import concourse.bass as bass
    import concourse.tile as tile
    from concourse import bacc, mybir

    f32 = mybir.dt.float32
    f16 = mybir.dt.float16
    SIG = mybir.ActivationFunctionType.Sigmoid

    klen_c = nkt * P          # compacted + padded klen
    blocks = []
    pos = 0
    while pos < klen_c:
        blocks.append((pos, min(512, klen_c - pos)))
        pos += 512
    nblk = len(blocks)

    nc = bacc.Bacc("TRN2", target_bir_lowering=False, debug=False,
                   num_devices=N_CORES)

    # Pre-blocked inputs: x[blk, p, c, l] = x_T[c*128+p, blk*512+l].
    qT_v = nc.dram_tensor("qT", [2, P, HID // P, 512], f16, kind="ExternalInput").ap()
    kT_v = nc.dram_tensor("kT", [nblk, P, HID // P, 512], f16, kind="ExternalInput").ap()
    vT_v = nc.dram_tensor("vT", [nblk, P, HID // P, 512], f16, kind="ExternalInput").ap()
    # weights as four contiguous 0.25MB blocks [wk0|wq0|wq1|wk1] so
    # every weight DMA is a whole contiguous block (strided sources run
    # at ~half the HBM rate)
    wkq_v = nc.dram_tensor("wkq", [4, P, HID // P, P], f16, kind="ExternalInput").ap()
    wvo_v = nc.dram_tensor("wvo", [2, P, 2 * HID], f16, kind="ExternalInput").ap()
    po_ap = nc.dram_tensor("po", [QLEN, HID], f16, kind="ExternalOutput").ap()
    dump_ap = nc.dram_tensor("dump", [1, 1], f16, kind="ExternalOutput").ap()

    NC_ = HID // P      # 8 contraction chunks

    with tile.TileContext(nc) as tc:
        with tc.tile_pool(name="sb", bufs=1) as sb, \
             tc.tile_pool(name="xin", bufs=2 * nblk + 2) as xin_pool, \
             tc.tile_pool(name="pt", bufs=20) as pt_pool, \
             tc.tile_pool(name="ost", bufs=4) as ost_pool, \
             tc.tile_pool(name="mm", bufs=2, space="PSUM") as mm_pool, \
             tc.tile_pool(name="av", bufs=2, space="PSUM") as av_pool, \
             tc.tile_pool(name="sps", bufs=2, space="PSUM") as s_pool:

            # ---- persistent tiles ----
            # [P, half(head-pair), kind(wq,wk), c, 128]
            wkq_sb = sb.tile([P, 2, 2, NC_, P], f16, tag="wkq")
            wvo_sb = sb.tile([P, 2, 2 * HID], f16, tag="wvo")

            v_sb = sb.tile([P, nkt, GSLICE], f16, tag="v")      # V natural [klen_c, 256]
            kt_sb = sb.tile([P, 2, klen_c], f16, tag="kt")      # K^T [hd(2x128), klen_c]
            qt_sb = sb.tile([P, 2, QLEN], f16, tag="qt")        # Q^T [hd, qlen]
            avt_sb = sb.tile([P, 2, 2, 512], f16, tag="avt")    # AV^T [hd, pair, qc, q]

            xq_t, xk_t, xv_t = {}, {}, {}

            # ---- DMA issue (order = priority = arrival urgency) ----
            def dma_x(store, dram, blk, chunks, nm=""):
                x = xin_pool.tile([P, NC_, 512], f16, tag="xin",
                                  name=f"x{nm}{blk}")
                blen = blocks[blk][1] if dram is not qT_v else 512
                for cc in range(0, NC_, chunks):
                    nc.sync.dma_start(out=x[:, cc:cc + chunks, 0:blen],
                                      in_=dram[blk, :, cc:cc + chunks, 0:blen])
                store[blk] = x

            nc.sync.dma_start(out=wkq_sb[:, 0, 1], in_=wkq_v[0])  # wk h0
            dma_x(xk_t, kT_v, 0, 8, "k")         # xk0 (one instr)
            nc.sync.dma_start(out=wkq_sb[:, 0, 0], in_=wkq_v[1])  # wq h0
            dma_x(xq_t, qT_v, 0, 4, "q")         # xq0 c0-3, c4-7
            for blk in range(2, nblk):
                dma_x(xk_t, kT_v, blk, 8, "k")   # xk2 (small tail block)
            nc.sync.dma_start(out=wkq_sb[:, 1, 0], in_=wkq_v[2])  # wq h1
            nc.sync.dma_start(out=wkq_sb[:, 1, 1], in_=wkq_v[3])  # wk h1
            dma_x(xk_t, kT_v, 1, 8, "k")         # xk1
            nc.sync.dma_start(out=wvo_sb[:, 0], in_=wvo_v[0])   # wv
            dma_x(xv_t, vT_v, 0, 8, "v")
            dma_x(xq_t, qT_v, 1, 8, "q")
            for blk in range(1, nblk):
                dma_x(xv_t, vT_v, blk, 8, "v")
            nc.sync.dma_start(out=wvo_sb[:, 1], in_=wvo_v[1])   # wo

            # ---- PE warm-up (keeps HAM at 2.4 GHz until real work) ----
            # Warmup matmuls go to an s_pool PSUM tile (not mm_pool) so
            # both mm bufs stay free for the first K/Q projections, and
            # more warmups can be interleaved into pre-spine DMA gaps.
            wtmp = sb.tile([P, 512], f16, tag="wtmp")
            nc.vector.memset(wtmp[:], 0.0)
            warm_ps = s_pool.tile([P, 2, 512], f32, tag="s", name="warm")

            def warm(n):
                for _ in range(n):
                    nc.tensor.matmul(warm_ps[:, 0, :], wtmp[:, 0:128],
                                     wtmp[:], start=True, stop=True)
            warm(13)

            # ---- attention primitives ----
            av_tiles = {}
            avt_done = set()

            def score(qc, pair, kt):
                sps = s_pool.tile([P, 2, 512], f32, tag="s",
                                  name=f"s{qc}_{pair}_{kt}")
                for h in range(2):
                    nc.tensor.matmul(
                        sps[:, h, :],
                        kt_sb[64 * h:64 * h + 64, pair, kt * P:(kt + 1) * P],
                        qt_sb[64 * h:64 * h + 64, pair, qc * 512:(qc + 1) * 512],
                        start=True, stop=True,
                    )
                return sps

            def sig(qc, pair, kt, sps):
                psb = pt_pool.tile([P, 2, 512], f16, tag="p",
                                   name=f"p{qc}_{pair}_{kt}")
                nc.scalar.activation(psb[:], sps[:], SIG, scale=float(SCALE))
                return psb

            def av(qc, pair, kt, psb, first, last):
                if (qc, pair) not in av_tiles:
                    av_tiles[(qc, pair)] = av_pool.tile(
                        [P, 512], f32, tag="av", name=f"av_{qc}_{pair}")
                avps = av_tiles[(qc, pair)]
                for h in range(2):
                    nc.tensor.matmul(
                        avps[64 * h:64 * h + 64, :],
                        v_sb[:, kt, pair * P + 64 * h: pair * P + 64 * h + 64],
                        psb[:, h, :],
                        start=first, stop=last,
                    )
                if last:
                    if (qc, pair) == (1, 1):
                        # final pass: per-qt casts so the tail out_proj
                        # matmuls start as soon as their column lands
                        for qt in range(4):
                            nc.vector.tensor_copy(
                                avt_sb[:, pair, qc, qt * P:(qt + 1) * P],
                                avps[:, qt * P:(qt + 1) * P])
                    else:
                        nc.vector.tensor_copy(avt_sb[:, pair, qc, :],
                                              avps[:])
                    del av_tiles[(qc, pair)]
                    avt_done.add((qc, pair))

            def op_nn(qc, qt, nn, ost_box, cast_eng):
                if '' not in ost_box:
                    ost_box[''] = ost_pool.tile([P, 2, 512], f16, tag="ost",
                                                name=f"os{qc}_{qt}")
                ost = ost_box['']
                ops = mm_pool.tile([P, 512], f32, tag="mm",
                                   name=f"o{qc}_{qt}_{nn}")
                for pr in range(2):
                    nc.tensor.matmul(
                        ops[:],
                        avt_sb[:, pr, qc, qt * P:(qt + 1) * P],
                        wvo_sb[:, 1, pr * HID + nn * 512:pr * HID + (nn + 1) * 512],
                        start=(pr == 0), stop=(pr == 1),
                    )
                if cast_eng == 's':
                    nc.scalar.copy(ost[:, nn, :], ops[:])
                else:
                    nc.vector.tensor_copy(ost[:, nn, :], ops[:])
                if nn == 1:
                    r0 = qc * 512 + qt * P
                    nc.sync.dma_start(out=po_ap[r0:r0 + P, :], in_=ost[:])

            # ---- virtual-time emission ----
            # Emission order defines Tile's dependencies and each
            # engine's runtime execution order.  A virtual PE clock is
            # advanced as instructions are emitted; queued atoms pop
            # between score pairs only while the projected PE time stays
            # below the point where the next score pair must start.
            # Deadlines are also the emission-order CORRECTNESS bound:
            # a producer atom must be emitted before the spine op that
            # reads its output (Tile deps follow emission order).
            passes = [(0, 0), (0, 1), (1, 0), (1, 1)]

            def tiles_of(b):
                t0 = blocks[b][0] // P
                return list(range(t0, t0 + blocks[b][1] // P))

            kt_nat = list(range(nkt))
            if nblk >= 3:
                # pass 0 visits the small tail block before block 1: its
                # tiny projection depends on xk2 which is DMA'd before
                # the big xk1, buying time for xk1 to land
                kt_seq0 = tiles_of(0) + tiles_of(nblk - 1)
                for b in range(1, nblk - 1):
                    kt_seq0 += tiles_of(b)
            else:
                kt_seq0 = kt_nat
            kt_seqs = [kt_seq0, kt_nat, kt_nat, kt_nat]
            slots = []
            for p, (qc, pair) in enumerate(passes):
                for si, kt in enumerate(kt_seqs[p]):
                    slots.append((qc, pair, kt, si == 0, si == nkt - 1))
            nslots = len(slots)

            def blk_of(kt):
                for bi in range(nblk):
                    t0 = blocks[bi][0] // P
                    if t0 <= kt < t0 + blocks[bi][1] // P:
                        return bi

            need_k, need_q = {}, {}
            for s_i, (qc, pair, kt, _f, _l) in enumerate(slots):
                need_k.setdefault((blk_of(kt), pair), s_i)
                need_q.setdefault((qc, pair), s_i)

            # estimated arrival (sem-complete) times: wire-serial at
            # ~2.9us/MB + ~0.3us per dma_start (16-engine completion
            # straggler), first data ~9.6us
            t_arr = {}
            _t = [9.4]

            def land(name, size_mb, n_instr=1):
                _t[0] += size_mb * 2.86 + 0.1 * n_instr
                t_arr[name] = _t[0]
            land('wk0', 0.25)
            land('xk0', blocks[0][1] / 512.)
            land('wq0', 0.25)
            land('xq0', 1., 2)
            for blk in range(2, nblk):
                land(f'xk{blk}', blocks[blk][1] / 512.)
            land('wq1', 0.25)
            land('wk1', 0.25)
            if nblk > 1:
                land('xk1', blocks[1][1] / 512.)
            land('wv', 0.5)
            land('xv0', blocks[0][1] / 512.)
            land('xq1', 1.)
            for blk in range(1, nblk):
                land(f'xv{blk}', blocks[blk][1] / 512.)
            land('wo', 0.5)

            # queue of atoms; mm-pool users carry a group id so at most
            # two PSUM tiles are ever live (bufs=2) -- a third alloc
            # would wait, at runtime, on a cast emitted after it (hang)
            queue = []
            state = {'open': [], 'vdone': set()}
            vt = {'pe': 0.0}

            def _pop(i):
                e = queue.pop(i)
                e['fn']()
                vt['pe'] = max(vt['pe'], e['rel']) + e['cost']
                gid = e.get('gid')
                if gid is not None:
                    if e.get('opens') and not e.get('closes'):
                        state['open'].append(gid)
                    if e.get('closes') and gid in state['open']:
                        state['open'].remove(gid)
                if e.get('vkt') is not None:
                    state['vdone'].add(e['vkt'])

            def _close_one():
                gid = state['open'][0]
                jj = next(j for j, e in enumerate(queue)
                          if e.get('gid') == gid)
                _pop(jj)

            def pump_until(tlimit, g):
                while True:
                    act = None
                    av_blk = False
                    for i, e in enumerate(queue):
                        forced = e['dl'] is not None and e['dl'] <= g
                        ready = e['rel'] <= vt['pe'] + 0.45
                        fits = vt['pe'] + e['cost'] <= tlimit
                        ok = forced or (ready and fits)
                        if e.get('av'):
                            blocked = av_blk
                            av_blk = True   # AV pops are strictly FIFO
                            if blocked:
                                continue
                            if e['vneed'] not in state['vdone']:
                                if forced:
                                    act = ('force_v', e['vneed'])
                                    break
                                continue
                            if ok:
                                act = ('pop', i)
                                break
                            continue
                        if e.get('navt') and not ({(0, 0), (0, 1)}
                                                  <= avt_done):
                            continue
                        if not ok:
                            continue
                        if e.get('opens') and e['gid'] not in state['open'] \
                                and len(state['open']) >= 2:
                            if forced:
                                act = ('close',)
                                break
                            continue
                        act = ('pop', i)
                        break
                    if act is None:
                        return
                    if act[0] == 'pop':
                        _pop(act[1])
                    elif act[0] == 'close':
                        _close_one()
                    else:
                        while len(state['open']) >= 2:
                            _close_one()
                        jj = next(j for j, e in enumerate(queue)
                                  if e.get('vkt') == act[1])
                        _pop(jj)

            # projection atoms -> queue (two ~0.9us pieces per half)
            def k_atoms(blk, half):
                dl = max(0, need_k[(blk, half)] - 2)
                rel = max(t_arr[f'xk{blk}'], t_arr[f'wk{half}'])
                pos, blen = blocks[blk]
                box = {}
                gid = ('k', blk, half)
                for piece in range(2):
                    def ap(piece=piece, blk=blk, half=half, box=box,
                           pos=pos, blen=blen):
                        if piece == 0:
                            box['t'] = mm_pool.tile(
                                [P, 512], f32, tag="mm",
                                name=f"kps{blk}_{half}")
                        kps = box['t']
                        for c in range(4 * piece, 4 * piece + 4):
                            nc.tensor.matmul(
                                kps[:, 0:blen],
                                wkq_sb[:, half, 1, c, :],
                                xk_t[blk][:, c, 0:blen],
                                start=(c == 0), stop=(c == NC_ - 1))
                        if piece == 1:
                            nc.vector.tensor_copy(
                                kt_sb[:, half, pos:pos + blen],
                                kps[:, 0:blen])
                    queue.append(dict(rel=rel, dl=dl, fn=ap,
                                      cost=0.9 * blen / 512, gid=gid,
                                      opens=(piece == 0),
                                      closes=(piece == 1)))

            def q_atoms(qblk, half):
                dl = max(0, need_q[(qblk, half)] - 2)
                rel = max(t_arr[f'xq{qblk}'], t_arr[f'wq{half}'])
                box = {}
                gid = ('q', qblk, half)
                for piece in range(2):
                    def ap(piece=piece, qblk=qblk, half=half, box=box):
                        if piece == 0:
                            box['t'] = mm_pool.tile(
                                [P, 512], f32, tag="mm",
                                name=f"qps{qblk}_{half}")
                        qps = box['t']
                        for c in range(4 * piece, 4 * piece + 4):
                            nc.tensor.matmul(
                                qps[:],
                                wkq_sb[:, half, 0, c, :],
                                xq_t[qblk][:, c, :],
                                start=(c == 0), stop=(c == NC_ - 1))
                        if piece == 1:
                            nc.vector.tensor_copy(
                                qt_sb[:, half, qblk * 512:(qblk + 1) * 512],
                                qps[:])
                    queue.append(dict(rel=rel, dl=dl, fn=ap, cost=0.9,
                                      gid=gid, opens=(piece == 0),
                                      closes=(piece == 1)))

            def v_atoms(blk):
                rel = max(t_arr[f'xv{blk}'], t_arr['wv'])
                pos, blen = blocks[blk]
                for j in range(blen // P):
                    kt_g = pos // P + j

                    def aj(blk=blk, j=j, kt_g=kt_g):
                        vps = mm_pool.tile([P, GSLICE], f32, tag="mm",
                                           name=f"vps{blk}_{j}")
                        for c in range(NC_):
                            nc.tensor.matmul(
                                vps[:],
                                xv_t[blk][:, c, j * P:(j + 1) * P],
                                wvo_sb[:, 0, c * GSLICE:(c + 1) * GSLICE],
                                start=(c == 0), stop=(c == NC_ - 1))
                        nc.vector.tensor_copy(v_sb[:, kt_g, :], vps[:])
                    queue.append(dict(rel=rel, dl=None, fn=aj, cost=0.9,
                                      gid=('v', blk, j), opens=True,
                                      closes=True, vkt=kt_g))

            # ---- spine ----
            # block-0 pair-0 K/Q projections gate slot 0 -- run direct,
            # warmup matmuls interleaved into the DMA-wait gaps so the
            # PE never idles long enough for a HAM MID-window rethrottle
            blen0 = blocks[0][1]
            kps0 = mm_pool.tile([P, 512], f32, tag="mm", name="kps0_0")
            for c in range(4):
                nc.tensor.matmul(kps0[:, 0:blen0], wkq_sb[:, 0, 1, c, :],
                                 xk_t[0][:, c, 0:blen0],
                                 start=(c == 0), stop=False)
            warm(2)
            for c in range(4, NC_):
                nc.tensor.matmul(kps0[:, 0:blen0], wkq_sb[:, 0, 1, c, :],
                                 xk_t[0][:, c, 0:blen0],
                                 start=False, stop=(c == NC_ - 1))
            nc.vector.tensor_copy(kt_sb[:, 0, 0:blen0], kps0[:, 0:blen0])
            warm(2)
            qps0 = mm_pool.tile([P, 512], f32, tag="mm", name="qps0_0")
            for c in range(4):
                nc.tensor.matmul(qps0[:], wkq_sb[:, 0, 0, c, :],
                                 xq_t[0][:, c, :],
                                 start=(c == 0), stop=False)
            warm(2)
            for c in range(4, NC_):
                nc.tensor.matmul(qps0[:], wkq_sb[:, 0, 0, c, :],
                                 xq_t[0][:, c, :],
                                 start=False, stop=(c == NC_ - 1))
            nc.vector.tensor_copy(qt_sb[:, 0, 0:512], qps0[:])
            # free the warmup PSUM tile; keep its result live via dump
            wsb = sb.tile([1, 1], f16, tag="wsb")
            nc.vector.tensor_copy(wsb[:], warm_ps[0:1, 0, 0:1])
            nc.sync.dma_start(out=dump_ap[0:1, 0:1], in_=wsb[:])
            # remaining projections as queue atoms, in rough need order
            for blk in range(1, nblk):
                k_atoms(blk, 0)
            k_atoms(0, 1)
            for blk in range(1, nblk):
                k_atoms(blk, 1)
            q_atoms(0, 1)
            q_atoms(1, 0)
            q_atoms(1, 1)
            for blk in range(nblk):
                v_atoms(blk)

            SIGD = 1.05          # sigmoid instruction + issue (us)
            L1, L2 = 0.12, 0.18  # score->sig and sig->bank-free handoff
            sig_end = {}
            vt['pe'] = t_arr['xq0'] + 1.5
            vt['act'] = 0.0
            sps_cur = score(*slots[0][:3])
            vt['pe'] += 0.42
            sc_done = vt['pe']
            for g, (qc, pair, kt, first, last) in enumerate(slots):
                psb = sig(qc, pair, kt, sps_cur)
                st = max(vt['act'], sc_done + L1)
                sig_end[g] = st + SIGD
                vt['act'] = sig_end[g]
                # AV atom: psb ready at sig end; V tile must be emitted
                # first (enforced via vneed); dl bounds the psb pool WAR
                vblk = blk_of(kt)
                vready = max(t_arr[f'xv{vblk}'], t_arr['wv']) + 1.2
                queue.append(dict(
                    rel=max(sig_end[g] + 0.1, vready), dl=g + 14,
                    fn=(lambda qc=qc, pair=pair, kt=kt, psb=psb,
                        first=first, last=last:
                        av(qc, pair, kt, psb, first, last)),
                    cost=0.38, av=True, vneed=kt))
                if (qc, pair) == (0, 1) and last:
                    avt01_t = sig_end[g] + 1.0
                    for qt in range(4):
                        box = {}
                        for nn in range(2):
                            queue.append(dict(
                                rel=avt01_t + qt * 0.4, dl=nslots - 8 + qt,
                                fn=(lambda qt=qt, nn=nn, box=box:
                                    op_nn(0, qt, nn, box, 'v')),
                                cost=0.5, gid=('o', qt, nn), opens=True,
                                closes=True, navt=True))
                if g + 1 < nslots:
                    bank_free = sig_end[g - 1] + L2 if g >= 1 else 0.0
                    target = max(sig_end[g] - 0.45, bank_free)
                    pump_until(target, g)
                    sps_cur = score(*slots[g + 1][:3])
                    vt['pe'] = max(vt['pe'], bank_free) + 0.42
                    sc_done = vt['pe']

            # ---- drain ----
            pump_until(1e9, nslots + 100)
            ost = ost_pool.tile([P, 2, 512], f16, tag="ost", name="os1_0")
            for nn in range(2):
                ops = mm_pool.tile([P, 512], f32, tag="mm",
                                   name=f"o1_0_{nn}")
                for pr in range(2):
                    nc.tensor.matmul(
                        ops[:], avt_sb[:, pr, 1, 0:P],
                        wvo_sb[:, 1,
                               pr * HID + nn * 512:pr * HID + (nn + 1) * 512],
                        start=(pr == 0), stop=(pr == 1))
                if nn == 0:
                    nc.vector.tensor_copy(ost[:, nn, :], ops[:])
                else:
                    nc.scalar.copy(ost[:, nn, :], ops[:])
            nc.sync.dma_start(out=po_ap[512:512 + P, :], in_=ost[:])
            # remaining out_proj(1) tiles: qt3 through mm_pool (freed by
            # op1 pr1 above) so its matmuls overlap qt1/qt2's evacuation
            box3 = {}
            op_nn(1, 3, 0, box3, 'v')
            op_nn(1, 3, 1, box3, 's')
            # qt1/qt2 in s_pool PSUM (free after the last sigmoid),
            # evac casts split across Vector/Scalar
            for qt in range(1, 3):
                osp = s_pool.tile([P, 2, 512], f32, tag="s",
                                  name=f"osp{qt}")
                for nn in range(2):
                    for pr in range(2):
                        nc.tensor.matmul(
                            osp[:, nn, :],
                            avt_sb[:, pr, 1, qt * P:(qt + 1) * P],
                            wvo_sb[:, 1, pr * HID + nn * 512:pr * HID + (nn + 1) * 512],
                            start=(pr == 0), stop=(pr == 1))
                ostq = ost_pool.tile([P, 2, 512], f16, tag="ost",
                                     name=f"os1_{qt}")
                nc.vector.tensor_copy(ostq[:, 0, :], osp[:, 0, :])
                nc.scalar.copy(ostq[:, 1, :], osp[:, 1, :])
                r0 = 512 + qt * P
                nc.sync.dma_start(out=po_ap[r0:r0 + P, :], in_=ostq[:])

    nc.compile()
    return nc


def _prep_in_maps(query, key, value, attn_mask, Wq, Wk, Wv, Wo):
    query = np.asarray(query, np.float32)
    key = np.asarray(key, np.float32)
    value = np.asarray(value, np.float32)
    mask = np.asarray(attn_mask)
    Wq = np.asarray(Wq, np.float32)
    Wk = np.asarray(Wk, np.float32)
    Wv = np.asarray(Wv, np.float32)
    Wo = np.asarray(Wo, np.float32)

    # Masked klen positions contribute exactly 0 (reference: sigmoid(-1e30)
    # == 0), so compact each batch to its unmasked positions, zero-padded
    # to a common multiple of 128.
    idxs = [np.nonzero(mask[b] != 0)[0] for b in range(BSZ)]
    klen_eff = max(len(ix) for ix in idxs)
    nkt = max(4, -(-klen_eff // P))
    klen_c = nkt * P

    nblk = (klen_c + 511) // 512
    klen_pad = nblk * 512

    def block_x(xT, width, pad_to):
        # [HID, width] -> [nblocks, 128, 8, 512] contiguous, zero-padded
        full = np.zeros((HID, pad_to), np.float16)
        full[:, :width] = xT
        nb = pad_to // 512
        return np.ascontiguousarray(
            full.reshape(HID // P, P, nb, 512).transpose(2, 1, 0, 3))

    kTc, vTc = [], []
    for b in range(BSZ):
        ix = idxs[b]
        kTc.append(block_x(key[b].T[:, ix].astype(np.float16), len(ix), klen_pad))
        vTc.append(block_x(value[b].T[:, ix].astype(np.float16), len(ix), klen_pad))

    qT0 = {}
    in_maps = []
    for core in range(N_CORES):
        b, g = divmod(core, 4)
        sl = slice(g * GSLICE, (g + 1) * GSLICE)
        if b not in qT0:
            qT0[b] = block_x(query[b].T.astype(np.float16), QLEN, QLEN)
        wq_h = (Wq[:, sl].astype(np.float16).reshape(HID // P, P, GSLICE)
                .transpose(1, 0, 2))
        wk_h = (Wk[:, sl].astype(np.float16).reshape(HID // P, P, GSLICE)
                .transpose(1, 0, 2))
        wv_h = (Wv[:, sl].astype(np.float16).reshape(HID // P, P, GSLICE)
                .transpose(1, 0, 2))
        wo_h = (Wo[sl, :].astype(np.float16).reshape(2, P, HID)
                .transpose(1, 0, 2))
        def wsplit(w_h, half):
            # [P, NC_, 256] -> [P, NC_, 128] for one head-pair half
            return w_h[:, :, half * P:(half + 1) * P]
        in_maps.append({
            "qT": qT0[b],
            "kT": kTc[b],
            "vT": vTc[b],
            "wkq": np.ascontiguousarray(
                np.stack([wsplit(wk_h, 0), wsplit(wq_h, 0),
                          wsplit(wq_h, 1), wsplit(wk_h, 1)])),
            "wvo": np.ascontiguousarray(
                np.stack([wv_h.reshape(P, 2 * HID),
                          wo_h.reshape(P, 2 * HID)])),
        })
    return in_maps, nkt


def _run(in_maps, nkt, trace):
    from concourse.bass_utils import run_bass_kernel_spmd

    if nkt not in _cache:
        _cache[nkt] = _build(nkt)
    res = run_bass_kernel_spmd(_cache[nkt], in_maps, list(range(N_CORES)),
                               trace=trace)
    out = np.zeros((BSZ, QLEN, HID), np.float32)
    for core in range(N_CORES):
        out[core // 4] += res.results[core]["po"].astype(np.float32)
    return out, res


def kernel(query, key, value, attn_mask, Wq, Wk, Wv, Wo):
    in_maps, nkt = _prep_in_maps(query, key, value, attn_mask, Wq, Wk, Wv, Wo)
    out, _ = _run(in_maps, nkt, trace=False)
    return out


def run_traced(query, key, value, attn_mask, Wq, Wk, Wv, Wo):
    """Like kernel() but with NTFF profiling; returns (out, exec_time_ns)."""
    in_maps, nkt = _prep_in_maps(query, key, value, attn_mask, Wq, Wk, Wv, Wo)
    out, res = _run(in_maps, nkt, trace=True)
    return out, res.exec_time_ns



# revision 36
# speedup vs baseline: 1.0506x; 1.0025x over previous
"""TRN2 Bass kernel for nn_MultiHeadAttn_1580547971654.

Multi-head attention with sigmoid activation (no softmax normalization),
2D key-side mask. query [2,1024,1024], key/value [2,2048,1024],
Wq/Wk/Wv [1024,1024], Wo [1024,1024], NH=16, HD=64.

Sharding (8 cores): data-parallel over batch (2) x tensor-parallel over
head groups (4 groups of 4 heads).  Core (b, g) computes
  partial[b] = sigmoid(scale * (q[b] Wq[:,G]) (k[b] Wk[:,G])^T) ((v[b]*mask) Wv[:,G]) Wo[G,:]
with G = head-group g's 256-wide hidden slice.  Host sums 4 partials per
batch.

Mask compaction: masked klen positions contribute exactly zero
(reference: sigmoid(-1e30) == 0), so the host gathers only unmasked
key/value columns, zero-padded to a multiple of 128.  With the uniform
0/1 mask this halves the klen-side work exactly.

Numerics: fp16 operands everywhere (TRN2 PE does native fp16 multiplies
with fp32 PSUM accumulation), so the only error is rounding tensors to
fp16 (2^-11).  Scale is folded into the sigmoid activation's scale.

Layout: activations are uploaded pre-transposed ([hidden, len]) so all
matmuls contract over the partition axis with no on-device transposes.
Per-head score matmuls (K=64) are row-packed in pairs into PE rows 0-63 /
64-127; attn@V matmuls (M=64) are col-packed in pairs.

Schedule: emission order defines both Tile's dependencies and each
engine's runtime execution order, so emission is driven by a virtual
PE clock.  The sigmoid stream is the spine (one score-pair lookahead +
sigmoid per slot); all other PE work (attn@V pairs, projections,
out-proj) is cut into ~0.4-0.9us atoms in a queue with release TIMES
from a calibrated DMA-arrival model (~9.4us first data + ~2.86us/MB,
wire-serial in issue order) and deadline slots that double as the
emission-order correctness bound (a reader must be emitted after its
producer).  pump_until() pops atoms between score pairs only while
the projected PE time stays below the next score's required start.
mm-pool PSUM tiles are guarded so at most two alloc+cast groups are
ever in flight (a third would deadlock the in-order PE behind a cast
emitted later); attn@V pops strictly FIFO with an explicit V-tile
emission dependency, trailing its sigmoid via a deep psb pool
(bufs=20).  DMA rules learned on HW: strided sources run at ~half
rate and every dma_start pays a ~0.5-1us 16-engine completion
straggler, so all weight blocks are host-packed contiguous
([wk0|wq0|wq1|wk1], [wv|wo]) and the critical-path stream is coarse:
wk0, xk0(1MB), wq0, xq0(2), xk2, wq1|wk1, xk1, wv, xv0, xq1, xv1,
xv2, wo.  Pass 0 visits the tiny tail block's kt before block 1 so
xk1's arrival is off the critical path.  PE warmup matmuls (into an
s_pool PSUM tile, keeping both mm bufs free) are interleaved into the
pre-spine DMA-wait gaps -- any ~2us+ PE idle window lets the HAM
clock-gate re-throttle the array to 1.2 GHz.  The tail finishes
out_proj(1) with per-qt avt casts, PSUM borrowed from the score pool,
and evac casts split across Vector/Scalar.
"""

import numpy as np

BSZ, QLEN, KLEN = 2, 1024, 2048
HID = 1024
NH, HD = 16, 64
SCALE = 1.0 / (HD ** 0.5)
N_CORES = 8
GSLICE = 256           # hidden slice per core (4 heads = 2 head-pairs)
P = 128

_cache = {}


def _build(nkt):
    import concourse.bass as bass
    import concourse.tile as tile
    from concourse import bacc, mybir

    f32 = mybir.dt.float32
    f16 = mybir.dt.float16
    SIG = mybir.ActivationFunctionType.Sigmoid

    klen_c = nkt * P          # compacted + padded klen
    blocks = []
    pos = 0
    while pos < klen_c:
        blocks.append((pos, min(512, klen_c - pos)))
        pos += 512
    nblk = len(blocks)

    nc = bacc.Bacc("TRN2", target_bir_lowering=False, debug=False,
                   num_devices=N_CORES)

    # Pre-blocked inputs: x[blk, p, c, l] = x_T[c*128+p, blk*512+l].
    qT_v = nc.dram_tensor("qT", [2, P, HID // P, 512], f16, kind="ExternalInput").ap()
    kT_v = nc.dram_tensor("kT", [nblk, P, HID // P, 512], f16, kind="ExternalInput").ap()
    vT_v = nc.dram_tensor("vT", [nblk, P, HID // P, 512], f16, kind="ExternalInput").ap()
    # weights as four contiguous 0.25MB blocks [wk0|wq0|wq1|wk1] so
    # every weight DMA is a whole contiguous block (strided sources run
    # at ~half the HBM rate)
    wkq_v = nc.dram_tensor("wkq", [4, P, HID // P, P], f16, kind="ExternalInput").ap()
    wvo_v = nc.dram_tensor("wvo", [2, P, 2 * HID], f16, kind="ExternalInput").ap()
    po_ap = nc.dram_tensor("po", [QLEN, HID], f16, kind="ExternalOutput").ap()
    dump_ap = nc.dram_tensor("dump", [1, 1], f16, kind="ExternalOutput").ap()

    NC_ = HID // P      # 8 contraction chunks

    with tile.TileContext(nc) as tc:
        with tc.tile_pool(name="sb", bufs=1) as sb, \
             tc.tile_pool(name="xin", bufs=2 * nblk + 2) as xin_pool, \
             tc.tile_pool(name="pt", bufs=20) as pt_pool, \
             tc.tile_pool(name="ost", bufs=4) as ost_pool, \
             tc.tile_pool(name="mm", bufs=2, space="PSUM") as mm_pool, \
             tc.tile_pool(name="av", bufs=2, space="PSUM") as av_pool, \
             tc.tile_pool(name="sps", bufs=2, space="PSUM") as s_pool:

            # ---- persistent tiles ----
            # [P, half(head-pair), kind(wq,wk), c, 128]
            wkq_sb = sb.tile([P, 2, 2, NC_, P], f16, tag="wkq")
            wvo_sb = sb.tile([P, 2, 2 * HID], f16, tag="wvo")

            v_sb = sb.tile([P, nkt, GSLICE], f16, tag="v")      # V natural [klen_c, 256]
            kt_sb = sb.tile([P, 2, klen_c], f16, tag="kt")      # K^T [hd(2x128), klen_c]
            qt_sb = sb.tile([P, 2, QLEN], f16, tag="qt")        # Q^T [hd, qlen]
            avt_sb = sb.tile([P, 2, 2, 512], f16, tag="avt")    # AV^T [hd, pair, qc, q]

            xq_t, xk_t, xv_t = {}, {}, {}

            # ---- DMA issue (order = priority = arrival urgency) ----
            def dma_x(store, dram, blk, chunks, nm=""):
                x = xin_pool.tile([P, NC_, 512], f16, tag="xin",
                                  name=f"x{nm}{blk}")
                blen = blocks[blk][1] if dram is not qT_v else 512
                for cc in range(0, NC_, chunks):
                    nc.sync.dma_start(out=x[:, cc:cc + chunks, 0:blen],
                                      in_=dram[blk, :, cc:cc + chunks, 0:blen])
                store[blk] = x

            nc.sync.dma_start(out=wkq_sb[:, 0, 1], in_=wkq_v[0])  # wk h0
            dma_x(xk_t, kT_v, 0, 8, "k")         # xk0 (one instr)
            nc.sync.dma_start(out=wkq_sb[:, 0, 0], in_=wkq_v[1])  # wq h0
            dma_x(xq_t, qT_v, 0, 4, "q")         # xq0 c0-3, c4-7
            for blk in range(2, nblk):
                dma_x(xk_t, kT_v, blk, 8, "k")   # xk2 (small tail block)
            nc.sync.dma_start(out=wkq_sb[:, 1, 0], in_=wkq_v[2])  # wq h1
            nc.sync.dma_start(out=wkq_sb[:, 1, 1], in_=wkq_v[3])  # wk h1
            dma_x(xk_t, kT_v, 1, 8, "k")         # xk1
            nc.sync.dma_start(out=wvo_sb[:, 0], in_=wvo_v[0])   # wv
            dma_x(xv_t, vT_v, 0, 8, "v")
            dma_x(xq_t, qT_v, 1, 8, "q")
            for blk in range(1, nblk):
                dma_x(xv_t, vT_v, blk, 8, "v")
            nc.sync.dma_start(out=wvo_sb[:, 1], in_=wvo_v[1])   # wo

            # ---- PE warm-up (keeps HAM at 2.4 GHz until real work) ----
            # Warmup matmuls go to an s_pool PSUM tile (not mm_pool) so
            # both mm bufs stay free for the first K/Q projections, and
            # more warmups can be interleaved into pre-spine DMA gaps.
            wtmp = sb.tile([P, 512], f16, tag="wtmp")
            nc.vector.memset(wtmp[:], 0.0)
            warm_ps = s_pool.tile([P, 2, 512], f32, tag="s", name="warm")

            def warm(n):
                for _ in range(n):
                    nc.tensor.matmul(warm_ps[:, 0, :], wtmp[:, 0:128],
                                     wtmp[:], start=True, stop=True)
            warm(13)

            # ---- attention primitives ----
            av_tiles = {}
            avt_done = set()

            def score(qc, pair, kt):
                sps = s_pool.tile([P, 2, 512], f32, tag="s",
                                  name=f"s{qc}_{pair}_{kt}")
                for h in range(2):
                    nc.tensor.matmul(
                        sps[:, h, :],
                        kt_sb[64 * h:64 * h + 64, pair, kt * P:(kt + 1) * P],
                        qt_sb[64 * h:64 * h + 64, pair, qc * 512:(qc + 1) * 512],
                        start=True, stop=True,
                    )
                return sps

            def sig(qc, pair, kt, sps):
                psb = pt_pool.tile([P, 2, 512], f16, tag="p",
                                   name=f"p{qc}_{pair}_{kt}")
                nc.scalar.activation(psb[:], sps[:], SIG, scale=float(SCALE))
                return psb

            def av(qc, pair, kt, psb, first, last):
                if (qc, pair) not in av_tiles:
                    av_tiles[(qc, pair)] = av_pool.tile(
                        [P, 512], f32, tag="av", name=f"av_{qc}_{pair}")
                avps = av_tiles[(qc, pair)]
                for h in range(2):
                    nc.tensor.matmul(
                        avps[64 * h:64 * h + 64, :],
                        v_sb[:, kt, pair * P + 64 * h: pair * P + 64 * h + 64],
                        psb[:, h, :],
                        start=first, stop=last,
                    )
                if last:
                    if (qc, pair) == (1, 1):
                        # final pass: per-qt casts so the tail out_proj
                        # matmuls start as soon as their column lands
                        for qt in range(4):
                            nc.vector.tensor_copy(
                                avt_sb[:, pair, qc, qt * P:(qt + 1) * P],
                                avps[:, qt * P:(qt + 1) * P])
                    else:
                        nc.vector.tensor_copy(avt_sb[:, pair, qc, :],
                                              avps[:])
                    del av_tiles[(qc, pair)]
                    avt_done.add((qc, pair))

            def op_nn(qc, qt, nn, ost_box, cast_eng):
                if '' not in ost_box:
                    ost_box[''] = ost_pool.tile([P, 2, 512], f16, tag="ost",
                                                name=f"os{qc}_{qt}")
                ost = ost_box['']
                ops = mm_pool.tile([P, 512], f32, tag="mm",
                                   name=f"o{qc}_{qt}_{nn}")
                for pr in range(2):
                    nc.tensor.matmul(
                        ops[:],
                        avt_sb[:, pr, qc, qt * P:(qt + 1) * P],
                        wvo_sb[:, 1, pr * HID + nn * 512:pr * HID + (nn + 1) * 512],
                        start=(pr == 0), stop=(pr == 1),
                    )
                if cast_eng == 's':
                    nc.scalar.copy(ost[:, nn, :], ops[:])
                else:
                    nc.vector.tensor_copy(ost[:, nn, :], ops[:])
                if nn == 1:
                    r0 = qc * 512 + qt * P
                    nc.sync.dma_start(out=po_ap[r0:r0 + P, :], in_=ost[:])

            # ---- virtual-time emission ----
            # Emission order defines Tile's dependencies and each
            # engine's runtime execution order.  A virtual PE clock is
            # advanced as instructions are emitted; queued atoms pop
            # between score pairs only while the projected PE time stays
            # below the point where the next score pair must start.
            # Deadlines are also the emission-order CORRECTNESS bound:
            # a producer atom must be emitted before the spine op that
            # reads its output (Tile deps follow emission order).
            passes = [(0, 0), (0, 1), (1, 0), (1, 1)]

            def tiles_of(b):
                t0 = blocks[b][0] // P
                return list(range(t0, t0 + blocks[b][1] // P))

            kt_nat = list(range(nkt))
            if nblk >= 3:
                # pass 0 visits the small tail block before block 1: its
                # tiny projection depends on xk2 which is DMA'd before
                # the big xk1, buying time for xk1 to land
                kt_seq0 = tiles_of(0) + tiles_of(nblk - 1)
                for b in range(1, nblk - 1):
                    kt_seq0 += tiles_of(b)
            else:
                kt_seq0 = kt_nat
            kt_seqs = [kt_seq0, kt_nat, kt_nat, kt_nat]
            slots = []
            for p, (qc, pair) in enumerate(passes):
                for si, kt in enumerate(kt_seqs[p]):
                    slots.append((qc, pair, kt, si == 0, si == nkt - 1))
            nslots = len(slots)

            def blk_of(kt):
                for bi in range(nblk):
                    t0 = blocks[bi][0] // P
                    if t0 <= kt < t0 + blocks[bi][1] // P:
                        return bi

            need_k, need_q = {}, {}
            for s_i, (qc, pair, kt, _f, _l) in enumerate(slots):
                need_k.setdefault((blk_of(kt), pair), s_i)
                need_q.setdefault((qc, pair), s_i)

            # estimated arrival (sem-complete) times: wire-serial at
            # ~2.9us/MB + ~0.3us per dma_start (16-engine completion
            # straggler), first data ~9.6us
            t_arr = {}
            _t = [9.4]

            def land(name, size_mb, n_instr=1):
                _t[0] += size_mb * 2.86 + 0.1 * n_instr
                t_arr[name] = _t[0]
            land('wk0', 0.25)
            land('xk0', blocks[0][1] / 512.)
            land('wq0', 0.25)
            land('xq0', 1., 2)
            for blk in range(2, nblk):
                land(f'xk{blk}', blocks[blk][1] / 512.)
            land('wq1', 0.25)
            land('wk1', 0.25)
            if nblk > 1:
                land('xk1', blocks[1][1] / 512.)
            land('wv', 0.5)
            land('xv0', blocks[0][1] / 512.)
            land('xq1', 1.)
            for blk in range(1, nblk):
                land(f'xv{blk}', blocks[blk][1] / 512.)
            land('wo', 0.5)

            # queue of atoms; mm-pool users carry a group id so at most
            # two PSUM tiles are ever live (bufs=2) -- a third alloc
            # would wait, at runtime, on a cast emitted after it (hang)
            queue = []
            state = {'open': [], 'vdone': set()}
            vt = {'pe': 0.0}

            def _pop(i):
                e = queue.pop(i)
                e['fn']()
                vt['pe'] = max(vt['pe'], e['rel']) + e['cost']
                gid = e.get('gid')
                if gid is not None:
                    if e.get('opens') and not e.get('closes'):
                        state['open'].append(gid)
                    if e.get('closes') and gid in state['open']:
                        state['open'].remove(gid)
                if e.get('vkt') is not None:
                    state['vdone'].add(e['vkt'])

            def _close_one():
                gid = state['open'][0]
                jj = next(j for j, e in enumerate(queue)
                          if e.get('gid') == gid)
                _pop(jj)

            def pump_until(tlimit, g):
                while True:
                    act = None
                    av_blk = False
                    for i, e in enumerate(queue):
                        forced = e['dl'] is not None and e['dl'] <= g
                        ready = e['rel'] <= vt['pe'] + 0.45
                        fits = vt['pe'] + e['cost'] <= tlimit
                        ok = forced or (ready and fits)
                        if e.get('av'):
                            blocked = av_blk
                            av_blk = True   # AV pops are strictly FIFO
                            if blocked:
                                continue
                            if e['vneed'] not in state['vdone']:
                                if forced:
                                    act = ('force_v', e['vneed'])
                                    break
                                continue
                            if ok:
                                act = ('pop', i)
                                break
                            continue
                        if e.get('navt') and not ({(0, 0), (0, 1)}
                                                  <= avt_done):
                            continue
                        if not ok:
                            continue
                        if e.get('opens') and e['gid'] not in state['open'] \
                                and len(state['open']) >= 2:
                            if forced:
                                act = ('close',)
                                break
                            continue
                        act = ('pop', i)
                        break
                    if act is None:
                        return
                    if act[0] == 'pop':
                        _pop(act[1])
                    elif act[0] == 'close':
                        _close_one()
                    else:
                        while len(state['open']) >= 2:
                            _close_one()
                        jj = next(j for j, e in enumerate(queue)
                                  if e.get('vkt') == act[1])
                        _pop(jj)

            # projection atoms -> queue (two ~0.9us pieces per half)
            def k_atoms(blk, half):
                dl = max(0, need_k[(blk, half)] - 2)
                rel = max(t_arr[f'xk{blk}'], t_arr[f'wk{half}'])
                pos, blen = blocks[blk]
                box = {}
                gid = ('k', blk, half)
                for piece in range(2):
                    def ap(piece=piece, blk=blk, half=half, box=box,
                           pos=pos, blen=blen):
                        if piece == 0:
                            box['t'] = mm_pool.tile(
                                [P, 512], f32, tag="mm",
                                name=f"kps{blk}_{half}")
                        kps = box['t']
                        for c in range(4 * piece, 4 * piece + 4):
                            nc.tensor.matmul(
                                kps[:, 0:blen],
                                wkq_sb[:, half, 1, c, :],
                                xk_t[blk][:, c, 0:blen],
                                start=(c == 0), stop=(c == NC_ - 1))
                        if piece == 1:
                            nc.vector.tensor_copy(
                                kt_sb[:, half, pos:pos + blen],
                                kps[:, 0:blen])
                    queue.append(dict(rel=rel, dl=dl, fn=ap,
                                      cost=0.9 * blen / 512, gid=gid,
                                      opens=(piece == 0),
                                      closes=(piece == 1)))

            def q_atoms(qblk, half):
                dl = max(0, need_q[(qblk, half)] - 2)
                rel = max(t_arr[f'xq{qblk}'], t_arr[f'wq{half}'])
                box = {}
                gid = ('q', qblk, half)
                for piece in range(2):
                    def ap(piece=piece, qblk=qblk, half=half, box=box):
                        if piece == 0:
                            box['t'] = mm_pool.tile(
                                [P, 512], f32, tag="mm",
                                name=f"qps{qblk}_{half}")
                        qps = box['t']
                        for c in range(4 * piece, 4 * piece + 4):
                            nc.tensor.matmul(
                                qps[:],
                                wkq_sb[:, half, 0, c, :],
                                xq_t[qblk][:, c, :],
                                start=(c == 0), stop=(c == NC_ - 1))
                        if piece == 1:
                            nc.vector.tensor_copy(
                                qt_sb[:, half, qblk * 512:(qblk + 1) * 512],
                                qps[:])
                    queue.append(dict(rel=rel, dl=dl, fn=ap, cost=0.9,
                                      gid=gid, opens=(piece == 0),
                                      closes=(piece == 1)))

            def v_atoms(blk):
                rel = max(t_arr[f'xv{blk}'], t_arr['wv'])
                pos, blen = blocks[blk]
                for j in range(blen // P):
                    kt_g = pos // P + j

                    def aj(blk=blk, j=j, kt_g=kt_g):
                        vps = mm_pool.tile([P, GSLICE], f32, tag="mm",
                                           name=f"vps{blk}_{j}")
                        for c in range(NC_):
                            nc.tensor.matmul(
                                vps[:],
                                xv_t[blk][:, c, j * P:(j + 1) * P],
                                wvo_sb[:, 0, c * GSLICE:(c + 1) * GSLICE],
                                start=(c == 0), stop=(c == NC_ - 1))
                        nc.vector.tensor_copy(v_sb[:, kt_g, :], vps[:])
                    queue.append(dict(rel=rel, dl=None, fn=aj, cost=0.9,
                                      gid=('v', blk, j), opens=True,
                                      closes=True, vkt=kt_g))

            # ---- spine ----
            # block-0 pair-0 K/Q projections gate slot 0 -- run direct,
            # warmup matmuls interleaved into the DMA-wait gaps so the
            # PE never idles long enough for a HAM MID-window rethrottle
            blen0 = blocks[0][1]
            kps0 = mm_pool.tile([P, 512], f32, tag="mm", name="kps0_0")
            for c in range(4):
                nc.tensor.matmul(kps0[:, 0:blen0], wkq_sb[:, 0, 1, c, :],
                                 xk_t[0][:, c, 0:blen0],
                                 start=(c == 0), stop=False)
            warm(2)
            for c in range(4, NC_):
                nc.tensor.matmul(kps0[:, 0:blen0], wkq_sb[:, 0, 1, c, :],
                                 xk_t[0][:, c, 0:blen0],
                                 start=False, stop=(c == NC_ - 1))
            nc.vector.tensor_copy(kt_sb[:, 0, 0:blen0], kps0[:, 0:blen0])
            warm(2)
            qps0 = mm_pool.tile([P, 512], f32, tag="mm", name="qps0_0")
            for c in range(4):
                nc.tensor.matmul(qps0[:], wkq_sb[:, 0, 0, c, :],
                                 xq_t[0][:, c, :],
                                 start=(c == 0), stop=False)
            warm(2)
            for c in range(4, NC_):
                nc.tensor.matmul(qps0[:], wkq_sb[:, 0, 0, c, :],
                                 xq_t[0][:, c, :],
                                 start=False, stop=(c == NC_ - 1))
            nc.vector.tensor_copy(qt_sb[:, 0, 0:512], qps0[:])
            # free the warmup PSUM tile; keep its result live via dump
            wsb = sb.tile([1, 1], f16, tag="wsb")
            nc.vector.tensor_copy(wsb[:], warm_ps[0:1, 0, 0:1])
            nc.sync.dma_start(out=dump_ap[0:1, 0:1], in_=wsb[:])
            # remaining projections as queue atoms, in rough need order
            for blk in range(1, nblk):
                k_atoms(blk, 0)
            k_atoms(0, 1)
            for blk in range(1, nblk):
                k_atoms(blk, 1)
            q_atoms(0, 1)
            q_atoms(1, 0)
            q_atoms(1, 1)
            for blk in range(nblk):
                v_atoms(blk)

            SIGD = 1.05          # sigmoid instruction + issue (us)
            L1, L2 = 0.12, 0.18  # score->sig and sig->bank-free handoff
            sig_end = {}
            vt['pe'] = t_arr['xq0'] + 1.5
            vt['act'] = 0.0
            sps_cur = score(*slots[0][:3])
            vt['pe'] += 0.42
            sc_done = vt['pe']
            for g, (qc, pair, kt, first, last) in enumerate(slots):
                psb = sig(qc, pair, kt, sps_cur)
                st = max(vt['act'], sc_done + L1)
                sig_end[g] = st + SIGD
                vt['act'] = sig_end[g]
                # AV atom: psb ready at sig end; V tile must be emitted
                # first (enforced via vneed); dl bounds the psb pool WAR
                vblk = blk_of(kt)
                vready = max(t_arr[f'xv{vblk}'], t_arr['wv']) + 1.2
                queue.append(dict(
                    rel=max(sig_end[g] + 0.1, vready), dl=g + 14,
                    fn=(lambda qc=qc, pair=pair, kt=kt, psb=psb,
                        first=first, last=last:
                        av(qc, pair, kt, psb, first, last)),
                    cost=0.38, av=True, vneed=kt))
                if (qc, pair) == (0, 1) and last:
                    avt01_t = sig_end[g] + 1.0
                    for qt in range(4):
                        box = {}
                        for nn in range(2):
                            queue.append(dict(
                                rel=avt01_t + qt * 0.4, dl=nslots - 8 + qt,
                                fn=(lambda qt=qt, nn=nn, box=box:
                                    op_nn(0, qt, nn, box, 'v')),
                                cost=0.5, gid=('o', qt, nn), opens=True,
                                closes=True, navt=True))
                if g + 1 < nslots:
                    bank_free = sig_end[g - 1] + L2 if g >= 1 else 0.0
                    target = max(sig_end[g] - 0.45, bank_free)
                    pump_until(target, g)
                    sps_cur = score(*slots[g + 1][:3])
                    vt['pe'] = max(vt['pe'], bank_free) + 0.42
                    sc_done = vt['pe']

            # ---- drain ----
            pump_until(1e9, nslots + 100)
            ost = ost_pool.tile([P, 2, 512], f16, tag="ost", name="os1_0")
            for nn in range(2):
                ops = mm_pool.tile([P, 512], f32, tag="mm",
                                   name=f"o1_0_{nn}")
                for pr in range(2):
                    nc.tensor.matmul(
                        ops[:], avt_sb[:, pr, 1, 0:P],
                        wvo_sb[:, 1,
                               pr * HID + nn * 512:pr * HID + (nn + 1) * 512],
                        start=(pr == 0), stop=(pr == 1))
                if nn == 0:
                    nc.vector.tensor_copy(ost[:, nn, :], ops[:])
                else:
                    nc.scalar.copy(ost[:, nn, :], ops[:])
            nc.sync.dma_start(out=po_ap[512:512 + P, :], in_=ost[:])
            # remaining out_proj(1) tiles: qt3 through mm_pool (freed by
            # op1 pr1 above) so its matmuls overlap qt1/qt2's evacuation
            box3 = {}
            op_nn(1, 3, 0, box3, 'v')
            op_nn(1, 3, 1, box3, 's')
            # qt1/qt2 in s_pool PSUM (free after the last sigmoid),
            # evac casts split across Vector/Scalar
            for qt in range(1, 3):
                osp = s_pool.tile([P, 2, 512], f32, tag="s",
                                  name=f"osp{qt}")
                for nn in range(2):
                    for pr in range(2):
                        nc.tensor.matmul(
                            osp[:, nn, :],
                            avt_sb[:, pr, 1, qt * P:(qt + 1) * P],
                            wvo_sb[:, 1, pr * HID + nn * 512:pr * HID + (nn + 1) * 512],
                            start=(pr == 0), stop=(pr == 1))
                ostq = ost_pool.tile([P, 2, 512], f16, tag="ost",
                                     name=f"os1_{qt}")
                nc.vector.tensor_copy(ostq[:, 0, :], osp[:, 0, :])
                nc.scalar.copy(ostq[:, 1, :], osp[:, 1, :])
                r0 = 512 + qt * P
                nc.sync.dma_start(out=po_ap[r0:r0 + P, :], in_=ostq[:])

    nc.compile()
    return nc


def _prep_in_maps(query, key, value, attn_mask, Wq, Wk, Wv, Wo):
    query = np.asarray(query, np.float32)
    key = np.asarray(key, np.float32)
    value = np.asarray(value, np.float32)
    mask = np.asarray(attn_mask)
    Wq = np.asarray(Wq, np.float32)
    Wk = np.asarray(Wk, np.float32)
    Wv = np.asarray(Wv, np.float32)
    Wo = np.asarray(Wo, np.float32)

    # Masked klen positions contribute exactly 0 (reference: sigmoid(-1e30)
    # == 0), so compact each batch to its unmasked positions, zero-padded
    # to a common multiple of 128.
    idxs = [np.nonzero(mask[b] != 0)[0] for b in range(BSZ)]
    klen_eff = max(len(ix) for ix in idxs)
    nkt = max(4, -(-klen_eff // P))
    klen_c = nkt * P

    nblk = (klen_c + 511) // 512
    klen_pad = nblk * 512

    def block_x(xT, width, pad_to):
        # [HID, width] -> [nblocks, 128, 8, 512] contiguous, zero-padded
        full = np.zeros((HID, pad_to), np.float16)
        full[:, :width] = xT
        nb = pad_to // 512
        return np.ascontiguousarray(
            full.reshape(HID // P, P, nb, 512).transpose(2, 1, 0, 3))

    kTc, vTc = [], []
    for b in range(BSZ):
        ix = idxs[b]
        kTc.append(block_x(key[b].T[:, ix].astype(np.float16), len(ix), klen_pad))
        vTc.append(block_x(value[b].T[:, ix].astype(np.float16), len(ix), klen_pad))

    qT0 = {}
    in_maps = []
    for core in range(N_CORES):
        b, g = divmod(core, 4)
        sl = slice(g * GSLICE, (g + 1) * GSLICE)
        if b not in qT0:
            qT0[b] = block_x(query[b].T.astype(np.float16), QLEN, QLEN)
        wq_h = (Wq[:, sl].astype(np.float16).reshape(HID // P, P, GSLICE)
                .transpose(1, 0, 2))
        wk_h = (Wk[:, sl].astype(np.float16).reshape(HID // P, P, GSLICE)
                .transpose(1, 0, 2))
        wv_h = (Wv[:, sl].astype(np.float16).reshape(HID // P, P, GSLICE)
                .transpose(1, 0, 2))
        wo_h = (Wo[sl, :].astype(np.float16).reshape(2, P, HID)
                .transpose(1, 0, 2))
        def wsplit(w_h, half):
            # [P, NC_, 256] -> [P, NC_, 128] for one head-pair half
            return w_h[:, :, half * P:(half + 1) * P]
        in_maps.append({
            "qT": qT0[b],
            "kT": kTc[b],
            "vT": vTc[b],
            "wkq": np.ascontiguousarray(
                np.stack([wsplit(wk_h, 0), wsplit(wq_h, 0),
                          wsplit(wq_h, 1), wsplit(wk_h, 1)])),
            "wvo": np.ascontiguousarray(
                np.stack([wv_h.reshape(P, 2 * HID),
                          wo_h.reshape(P, 2 * HID)])),
        })
    return in_maps, nkt


def _run(in_maps, nkt, trace):
    from concourse.bass_utils import run_bass_kernel_spmd

    if nkt not in _cache:
        _cache[nkt] = _build(nkt)
    res = run_bass_kernel_spmd(_cache[nkt], in_maps, list(range(N_CORES)),
                               trace=trace)
    out = np.zeros((BSZ, QLEN, HID), np.float32)
    for core in range(N_CORES):
        out[core // 4] += res.results[core]["po"].astype(np.float32)
    return out, res


def kernel(query, key, value, attn_mask, Wq, Wk, Wv, Wo):
    in_maps, nkt = _prep_in_maps(query, key, value, attn_mask, Wq, Wk, Wv, Wo)
    out, _ = _run(in_maps, nkt, trace=False)
    return out


def run_traced(query, key, value, attn_mask, Wq, Wk, Wv, Wo):
    """Like kernel() but with NTFF profiling; returns (out, exec_time_ns)."""
    in_maps, nkt = _prep_in_maps(query, key, value, attn_mask, Wq, Wk, Wv, Wo)
    out, res = _run(in_maps, nkt, trace=True)
    return out, res.exec_time_ns



# revision 37
# speedup vs baseline: 1.0512x; 1.0005x over previous
"""TRN2 Bass kernel for nn_MultiHeadAttn_1580547971654.

Multi-head attention with sigmoid activation (no softmax normalization),
2D key-side mask. query [2,1024,1024], key/value [2,2048,1024],
Wq/Wk/Wv [1024,1024], Wo [1024,1024], NH=16, HD=64.

Sharding (8 cores): data-parallel over batch (2) x tensor-parallel over
head groups (4 groups of 4 heads).  Core (b, g) computes
  partial[b] = sigmoid(scale * (q[b] Wq[:,G]) (k[b] Wk[:,G])^T) ((v[b]*mask) Wv[:,G]) Wo[G,:]
with G = head-group g's 256-wide hidden slice.  Host sums 4 partials per
batch.

Mask compaction: masked klen positions contribute exactly zero
(reference: sigmoid(-1e30) == 0), so the host gathers only unmasked
key/value columns, zero-padded to a multiple of 128.  With the uniform
0/1 mask this halves the klen-side work exactly.

Numerics: fp16 operands everywhere (TRN2 PE does native fp16 multiplies
with fp32 PSUM accumulation), so the only error is rounding tensors to
fp16 (2^-11).  Scale is folded into the sigmoid activation's scale.

Layout: activations are uploaded pre-transposed ([hidden, len]) so all
matmuls contract over the partition axis with no on-device transposes.
Per-head score matmuls (K=64) are row-packed in pairs into PE rows 0-63 /
64-127; attn@V matmuls (M=64) are col-packed in pairs.

Schedule: emission order defines both Tile's dependencies and each
engine's runtime execution order, so emission is driven by a virtual
PE clock.  The sigmoid stream is the spine (one score-pair lookahead +
sigmoid per slot); all other PE work (attn@V pairs, projections,
out-proj) is cut into ~0.4-0.9us atoms in a queue with release TIMES
from a calibrated DMA-arrival model (~9.4us first data + ~2.86us/MB,
wire-serial in issue order) and deadline slots that double as the
emission-order correctness bound (a reader must be emitted after its
producer).  pump_until() pops atoms between score pairs only while
the projected PE time stays below the next score's required start.
mm-pool PSUM tiles are guarded so at most two alloc+cast groups are
ever in flight (a third would deadlock the in-order PE behind a cast
emitted later); attn@V pops strictly FIFO with an explicit V-tile
emission dependency, trailing its sigmoid via a deep psb pool
(bufs=20).  DMA rules learned on HW: strided sources run at ~half
rate and every dma_start pays a ~0.5-1us 16-engine completion
straggler, so all weight blocks are host-packed contiguous
([wk0|wq0|wq1|wk1], [wv|wo]) and the critical-path stream is coarse:
wk0, xk0(1MB), wq0, xq0(2), xk2, wq1|wk1, xk1, wv, xv0, xq1, xv1,
xv2, wo.  Pass 0 visits the tiny tail block's kt before block 1 so
xk1's arrival is off the critical path.  PE warmup matmuls (into an
s_pool PSUM tile, keeping both mm bufs free) are interleaved into the
pre-spine DMA-wait gaps -- any ~2us+ PE idle window lets the HAM
clock-gate re-throttle the array to 1.2 GHz.  The tail finishes
out_proj(1) with per-qt avt casts, PSUM borrowed from the score pool,
and evac casts split across Vector/Scalar.
"""

import numpy as np

BSZ, QLEN, KLEN = 2, 1024, 2048
HID = 1024
NH, HD = 16, 64
SCALE = 1.0 / (HD ** 0.5)
N_CORES = 8
GSLICE = 256           # hidden slice per core (4 heads = 2 head-pairs)
P = 128

_cache = {}


def _build(nkt):
    import concourse.bass as bass
    import concourse.tile as tile
    from concourse import bacc, mybir

    f32 = mybir.dt.float32
    f16 = mybir.dt.float16
    SIG = mybir.ActivationFunctionType.Sigmoid

    klen_c = nkt * P          # compacted + padded klen
    blocks = []
    pos = 0
    while pos < klen_c:
        blocks.append((pos, min(512, klen_c - pos)))
        pos += 512
    nblk = len(blocks)

    nc = bacc.Bacc("TRN2", target_bir_lowering=False, debug=False,
                   num_devices=N_CORES)

    # Pre-blocked inputs: x[blk, p, c, l] = x_T[c*128+p, blk*512+l].
    qT_v = nc.dram_tensor("qT", [2, P, HID // P, 512], f16, kind="ExternalInput").ap()
    kT_v = nc.dram_tensor("kT", [nblk, P, HID // P, 512], f16, kind="ExternalInput").ap()
    vT_v = nc.dram_tensor("vT", [nblk, P, HID // P, 512], f16, kind="ExternalInput").ap()
    # weights as four contiguous 0.25MB blocks [wk0|wq0|wq1|wk1] so
    # every weight DMA is a whole contiguous block (strided sources run
    # at ~half the HBM rate)
    wkq_v = nc.dram_tensor("wkq", [4, P, HID // P, P], f16, kind="ExternalInput").ap()
    wvo_v = nc.dram_tensor("wvo", [2, P, 2 * HID], f16, kind="ExternalInput").ap()
    po_ap = nc.dram_tensor("po", [QLEN, HID], f16, kind="ExternalOutput").ap()
    dump_ap = nc.dram_tensor("dump", [1, 1], f16, kind="ExternalOutput").ap()

    NC_ = HID // P      # 8 contraction chunks

    with tile.TileContext(nc) as tc:
        with tc.tile_pool(name="sb", bufs=1) as sb, \
             tc.tile_pool(name="xin", bufs=2 * nblk + 2) as xin_pool, \
             tc.tile_pool(name="pt", bufs=20) as pt_pool, \
             tc.tile_pool(name="ost", bufs=4) as ost_pool, \
             tc.tile_pool(name="mm", bufs=2, space="PSUM") as mm_pool, \
             tc.tile_pool(name="av", bufs=2, space="PSUM") as av_pool, \
             tc.tile_pool(name="sps", bufs=2, space="PSUM") as s_pool:

            # ---- persistent tiles ----
            # [P, half(head-pair), kind(wq,wk), c, 128]
            wkq_sb = sb.tile([P, 2, 2, NC_, P], f16, tag="wkq")
            wvo_sb = sb.tile([P, 2, 2 * HID], f16, tag="wvo")

            v_sb = sb.tile([P, nkt, GSLICE], f16, tag="v")      # V natural [klen_c, 256]
            kt_sb = sb.tile([P, 2, klen_c], f16, tag="kt")      # K^T [hd(2x128), klen_c]
            qt_sb = sb.tile([P, 2, QLEN], f16, tag="qt")        # Q^T [hd, qlen]
            avt_sb = sb.tile([P, 2, 2, 512], f16, tag="avt")    # AV^T [hd, pair, qc, q]

            xq_t, xk_t, xv_t = {}, {}, {}

            # ---- DMA issue (order = priority = arrival urgency) ----
            def dma_x(store, dram, blk, chunks, nm=""):
                x = xin_pool.tile([P, NC_, 512], f16, tag="xin",
                                  name=f"x{nm}{blk}")
                blen = blocks[blk][1] if dram is not qT_v else 512
                for cc in range(0, NC_, chunks):
                    nc.sync.dma_start(out=x[:, cc:cc + chunks, 0:blen],
                                      in_=dram[blk, :, cc:cc + chunks, 0:blen])
                store[blk] = x

            nc.sync.dma_start(out=wkq_sb[:, 0, 1], in_=wkq_v[0])  # wk h0
            dma_x(xk_t, kT_v, 0, 8, "k")         # xk0 (one instr)
            nc.sync.dma_start(out=wkq_sb[:, 0, 0], in_=wkq_v[1])  # wq h0
            dma_x(xq_t, qT_v, 0, 4, "q")         # xq0 c0-3, c4-7
            for blk in range(2, nblk):
                dma_x(xk_t, kT_v, blk, 8, "k")   # xk2 (small tail block)
            nc.sync.dma_start(out=wkq_sb[:, 1, 0], in_=wkq_v[2])  # wq h1
            nc.sync.dma_start(out=wkq_sb[:, 1, 1], in_=wkq_v[3])  # wk h1
            dma_x(xk_t, kT_v, 1, 8, "k")         # xk1
            nc.sync.dma_start(out=wvo_sb[:, 0], in_=wvo_v[0])   # wv
            dma_x(xv_t, vT_v, 0, 8, "v")
            dma_x(xq_t, qT_v, 1, 8, "q")
            for blk in range(1, nblk):
                dma_x(xv_t, vT_v, blk, 8, "v")
            nc.sync.dma_start(out=wvo_sb[:, 1], in_=wvo_v[1])   # wo

            # ---- PE warm-up (keeps HAM at 2.4 GHz until real work) ----
            # Warmup matmuls go to an s_pool PSUM tile (not mm_pool) so
            # both mm bufs stay free for the first K/Q projections, and
            # more warmups can be interleaved into pre-spine DMA gaps.
            wtmp = sb.tile([P, 512], f16, tag="wtmp")
            nc.vector.memset(wtmp[:], 0.0)
            warm_ps = s_pool.tile([P, 2, 512], f32, tag="s", name="warm")

            def warm(n):
                for _ in range(n):
                    nc.tensor.matmul(warm_ps[:, 0, :], wtmp[:, 0:128],
                                     wtmp[:], start=True, stop=True)
            warm(13)

            # ---- attention primitives ----
            av_tiles = {}
            avt_done = set()

            def score(qc, pair, kt):
                sps = s_pool.tile([P, 2, 512], f32, tag="s",
                                  name=f"s{qc}_{pair}_{kt}")
                for h in range(2):
                    nc.tensor.matmul(
                        sps[:, h, :],
                        kt_sb[64 * h:64 * h + 64, pair, kt * P:(kt + 1) * P],
                        qt_sb[64 * h:64 * h + 64, pair, qc * 512:(qc + 1) * 512],
                        start=True, stop=True,
                    )
                return sps

            def sig(qc, pair, kt, sps):
                psb = pt_pool.tile([P, 2, 512], f16, tag="p",
                                   name=f"p{qc}_{pair}_{kt}")
                nc.scalar.activation(psb[:], sps[:], SIG, scale=float(SCALE))
                return psb

            def av(qc, pair, kt, psb, first, last):
                if (qc, pair) not in av_tiles:
                    av_tiles[(qc, pair)] = av_pool.tile(
                        [P, 512], f32, tag="av", name=f"av_{qc}_{pair}")
                avps = av_tiles[(qc, pair)]
                for h in range(2):
                    nc.tensor.matmul(
                        avps[64 * h:64 * h + 64, :],
                        v_sb[:, kt, pair * P + 64 * h: pair * P + 64 * h + 64],
                        psb[:, h, :],
                        start=first, stop=last,
                    )
                if last:
                    if (qc, pair) == (1, 1):
                        # final pass: per-qt casts so the tail out_proj
                        # matmuls start as soon as their column lands
                        for qt in range(4):
                            nc.vector.tensor_copy(
                                avt_sb[:, pair, qc, qt * P:(qt + 1) * P],
                                avps[:, qt * P:(qt + 1) * P])
                    else:
                        nc.vector.tensor_copy(avt_sb[:, pair, qc, :],
                                              avps[:])
                    del av_tiles[(qc, pair)]
                    avt_done.add((qc, pair))

            def op_nn(qc, qt, nn, ost_box, cast_eng):
                if '' not in ost_box:
                    ost_box[''] = ost_pool.tile([P, 2, 512], f16, tag="ost",
                                                name=f"os{qc}_{qt}")
                ost = ost_box['']
                ops = mm_pool.tile([P, 512], f32, tag="mm",
                                   name=f"o{qc}_{qt}_{nn}")
                for pr in range(2):
                    nc.tensor.matmul(
                        ops[:],
                        avt_sb[:, pr, qc, qt * P:(qt + 1) * P],
                        wvo_sb[:, 1, pr * HID + nn * 512:pr * HID + (nn + 1) * 512],
                        start=(pr == 0), stop=(pr == 1),
                    )
                if cast_eng == 's':
                    nc.scalar.copy(ost[:, nn, :], ops[:])
                else:
                    nc.vector.tensor_copy(ost[:, nn, :], ops[:])
                if nn == 1:
                    r0 = qc * 512 + qt * P
                    nc.sync.dma_start(out=po_ap[r0:r0 + P, :], in_=ost[:])

            # ---- virtual-time emission ----
            # Emission order defines Tile's dependencies and each
            # engine's runtime execution order.  A virtual PE clock is
            # advanced as instructions are emitted; queued atoms pop
            # between score pairs only while the projected PE time stays
            # below the point where the next score pair must start.
            # Deadlines are also the emission-order CORRECTNESS bound:
            # a producer atom must be emitted before the spine op that
            # reads its output (Tile deps follow emission order).
            passes = [(0, 0), (0, 1), (1, 0), (1, 1)]

            def tiles_of(b):
                t0 = blocks[b][0] // P
                return list(range(t0, t0 + blocks[b][1] // P))

            kt_nat = list(range(nkt))
            if nblk >= 3:
                # pass 0 visits the small tail block before block 1: its
                # tiny projection depends on xk2 which is DMA'd before
                # the big xk1, buying time for xk1 to land
                kt_seq0 = tiles_of(0) + tiles_of(nblk - 1)
                for b in range(1, nblk - 1):
                    kt_seq0 += tiles_of(b)
            else:
                kt_seq0 = kt_nat
            kt_seqs = [kt_seq0, kt_nat, kt_nat, kt_nat]
            slots = []
            for p, (qc, pair) in enumerate(passes):
                for si, kt in enumerate(kt_seqs[p]):
                    slots.append((qc, pair, kt, si == 0, si == nkt - 1))
            nslots = len(slots)

            def blk_of(kt):
                for bi in range(nblk):
                    t0 = blocks[bi][0] // P
                    if t0 <= kt < t0 + blocks[bi][1] // P:
                        return bi

            need_k, need_q = {}, {}
            for s_i, (qc, pair, kt, _f, _l) in enumerate(slots):
                need_k.setdefault((blk_of(kt), pair), s_i)
                need_q.setdefault((qc, pair), s_i)

            # estimated arrival (sem-complete) times: wire-serial at
            # ~2.9us/MB + ~0.3us per dma_start (16-engine completion
            # straggler), first data ~9.6us
            t_arr = {}
            _t = [9.4]

            def land(name, size_mb, n_instr=1):
                _t[0] += size_mb * 2.86 + 0.1 * n_instr
                t_arr[name] = _t[0]
            land('wk0', 0.25)
            land('xk0', blocks[0][1] / 512.)
            land('wq0', 0.25)
            land('xq0', 1., 2)
            for blk in range(2, nblk):
                land(f'xk{blk}', blocks[blk][1] / 512.)
            land('wq1', 0.25)
            land('wk1', 0.25)
            if nblk > 1:
                land('xk1', blocks[1][1] / 512.)
            land('wv', 0.5)
            land('xv0', blocks[0][1] / 512.)
            land('xq1', 1.)
            for blk in range(1, nblk):
                land(f'xv{blk}', blocks[blk][1] / 512.)
            land('wo', 0.5)

            # queue of atoms; mm-pool users carry a group id so at most
            # two PSUM tiles are ever live (bufs=2) -- a third alloc
            # would wait, at runtime, on a cast emitted after it (hang)
            queue = []
            state = {'open': [], 'vdone': set()}
            vt = {'pe': 0.0}

            def _pop(i):
                e = queue.pop(i)
                e['fn']()
                vt['pe'] = max(vt['pe'], e['rel']) + e['cost']
                gid = e.get('gid')
                if gid is not None:
                    if e.get('opens') and not e.get('closes'):
                        state['open'].append(gid)
                    if e.get('closes') and gid in state['open']:
                        state['open'].remove(gid)
                if e.get('vkt') is not None:
                    state['vdone'].add(e['vkt'])

            def _close_one():
                gid = state['open'][0]
                jj = next(j for j, e in enumerate(queue)
                          if e.get('gid') == gid)
                _pop(jj)

            def pump_until(tlimit, g):
                while True:
                    act = None
                    av_blk = False
                    for i, e in enumerate(queue):
                        forced = e['dl'] is not None and e['dl'] <= g
                        ready = e['rel'] <= vt['pe'] + 0.45
                        fits = vt['pe'] + e['cost'] <= tlimit
                        ok = forced or (ready and fits)
                        if e.get('av'):
                            blocked = av_blk
                            av_blk = True   # AV pops are strictly FIFO
                            if blocked:
                                continue
                            if e['vneed'] not in state['vdone']:
                                if forced:
                                    act = ('force_v', e['vneed'])
                                    break
                                continue
                            if ok:
                                act = ('pop', i)
                                break
                            continue
                        if e.get('navt') and not ({(0, 0), (0, 1)}
                                                  <= avt_done):
                            continue
                        if not ok:
                            continue
                        if e.get('opens') and e['gid'] not in state['open'] \
                                and len(state['open']) >= 2:
                            if forced:
                                act = ('close',)
                                break
                            continue
                        act = ('pop', i)
                        break
                    if act is None:
                        return
                    if act[0] == 'pop':
                        _pop(act[1])
                    elif act[0] == 'close':
                        _close_one()
                    else:
                        while len(state['open']) >= 2:
                            _close_one()
                        jj = next(j for j, e in enumerate(queue)
                                  if e.get('vkt') == act[1])
                        _pop(jj)

            # projection atoms -> queue (two ~0.9us pieces per half)
            def k_atoms(blk, half):
                dl = max(0, need_k[(blk, half)] - 1)
                rel = max(t_arr[f'xk{blk}'], t_arr[f'wk{half}'])
                pos, blen = blocks[blk]
                box = {}
                gid = ('k', blk, half)
                for piece in range(2):
                    def ap(piece=piece, blk=blk, half=half, box=box,
                           pos=pos, blen=blen):
                        if piece == 0:
                            box['t'] = mm_pool.tile(
                                [P, 512], f32, tag="mm",
                                name=f"kps{blk}_{half}")
                        kps = box['t']
                        for c in range(4 * piece, 4 * piece + 4):
                            nc.tensor.matmul(
                                kps[:, 0:blen],
                                wkq_sb[:, half, 1, c, :],
                                xk_t[blk][:, c, 0:blen],
                                start=(c == 0), stop=(c == NC_ - 1))
                        if piece == 1:
                            nc.vector.tensor_copy(
                                kt_sb[:, half, pos:pos + blen],
                                kps[:, 0:blen])
                    queue.append(dict(rel=rel, dl=dl, fn=ap,
                                      cost=0.9 * blen / 512, gid=gid,
                                      opens=(piece == 0),
                                      closes=(piece == 1)))

            def q_atoms(qblk, half):
                dl = max(0, need_q[(qblk, half)] - 1)
                rel = max(t_arr[f'xq{qblk}'], t_arr[f'wq{half}'])
                box = {}
                gid = ('q', qblk, half)
                for piece in range(2):
                    def ap(piece=piece, qblk=qblk, half=half, box=box):
                        if piece == 0:
                            box['t'] = mm_pool.tile(
                                [P, 512], f32, tag="mm",
                                name=f"qps{qblk}_{half}")
                        qps = box['t']
                        for c in range(4 * piece, 4 * piece + 4):
                            nc.tensor.matmul(
                                qps[:],
                                wkq_sb[:, half, 0, c, :],
                                xq_t[qblk][:, c, :],
                                start=(c == 0), stop=(c == NC_ - 1))
                        if piece == 1:
                            nc.vector.tensor_copy(
                                qt_sb[:, half, qblk * 512:(qblk + 1) * 512],
                                qps[:])
                    queue.append(dict(rel=rel, dl=dl, fn=ap, cost=0.9,
                                      gid=gid, opens=(piece == 0),
                                      closes=(piece == 1)))

            def v_atoms(blk):
                rel = max(t_arr[f'xv{blk}'], t_arr['wv'])
                pos, blen = blocks[blk]
                for j in range(blen // P):
                    kt_g = pos // P + j

                    def aj(blk=blk, j=j, kt_g=kt_g):
                        vps = mm_pool.tile([P, GSLICE], f32, tag="mm",
                                           name=f"vps{blk}_{j}")
                        for c in range(NC_):
                            nc.tensor.matmul(
                                vps[:],
                                xv_t[blk][:, c, j * P:(j + 1) * P],
                                wvo_sb[:, 0, c * GSLICE:(c + 1) * GSLICE],
                                start=(c == 0), stop=(c == NC_ - 1))
                        nc.vector.tensor_copy(v_sb[:, kt_g, :], vps[:])
                    queue.append(dict(rel=rel, dl=None, fn=aj, cost=0.9,
                                      gid=('v', blk, j), opens=True,
                                      closes=True, vkt=kt_g))

            # ---- spine ----
            # block-0 pair-0 K/Q projections gate slot 0 -- run direct,
            # warmup matmuls interleaved into the DMA-wait gaps so the
            # PE never idles long enough for a HAM MID-window rethrottle
            blen0 = blocks[0][1]
            kps0 = mm_pool.tile([P, 512], f32, tag="mm", name="kps0_0")
            for c in range(4):
                nc.tensor.matmul(kps0[:, 0:blen0], wkq_sb[:, 0, 1, c, :],
                                 xk_t[0][:, c, 0:blen0],
                                 start=(c == 0), stop=False)
            warm(2)
            for c in range(4, NC_):
                nc.tensor.matmul(kps0[:, 0:blen0], wkq_sb[:, 0, 1, c, :],
                                 xk_t[0][:, c, 0:blen0],
                                 start=False, stop=(c == NC_ - 1))
            nc.vector.tensor_copy(kt_sb[:, 0, 0:blen0], kps0[:, 0:blen0])
            warm(2)
            qps0 = mm_pool.tile([P, 512], f32, tag="mm", name="qps0_0")
            for c in range(4):
                nc.tensor.matmul(qps0[:], wkq_sb[:, 0, 0, c, :],
                                 xq_t[0][:, c, :],
                                 start=(c == 0), stop=False)
            warm(2)
            for c in range(4, NC_):
                nc.tensor.matmul(qps0[:], wkq_sb[:, 0, 0, c, :],
                                 xq_t[0][:, c, :],
                                 start=False, stop=(c == NC_ - 1))
            nc.vector.tensor_copy(qt_sb[:, 0, 0:512], qps0[:])
            # free the warmup PSUM tile; keep its result live via dump
            wsb = sb.tile([1, 1], f16, tag="wsb")
            nc.vector.tensor_copy(wsb[:], warm_ps[0:1, 0, 0:1])
            nc.sync.dma_start(out=dump_ap[0:1, 0:1], in_=wsb[:])
            # remaining projections as queue atoms, in rough need order
            for blk in range(1, nblk):
                k_atoms(blk, 0)
            k_atoms(0, 1)
            for blk in range(1, nblk):
                k_atoms(blk, 1)
            q_atoms(0, 1)
            q_atoms(1, 0)
            q_atoms(1, 1)
            for blk in range(nblk):
                v_atoms(blk)

            SIGD = 1.0           # sigmoid instruction + issue (us)
            L1, L2 = 0.12, 0.18  # score->sig and sig->bank-free handoff
            sig_end = {}
            vt['pe'] = t_arr['xq0'] + 1.5
            vt['act'] = 0.0
            sps_cur = score(*slots[0][:3])
            vt['pe'] += 0.42
            sc_done = vt['pe']
            for g, (qc, pair, kt, first, last) in enumerate(slots):
                psb = sig(qc, pair, kt, sps_cur)
                st = max(vt['act'], sc_done + L1)
                sig_end[g] = st + SIGD
                vt['act'] = sig_end[g]
                # AV atom: psb ready at sig end; V tile must be emitted
                # first (enforced via vneed); dl bounds the psb pool WAR
                vblk = blk_of(kt)
                vready = max(t_arr[f'xv{vblk}'], t_arr['wv']) + 1.2
                queue.append(dict(
                    rel=max(sig_end[g] + 0.1, vready), dl=g + 14,
                    fn=(lambda qc=qc, pair=pair, kt=kt, psb=psb,
                        first=first, last=last:
                        av(qc, pair, kt, psb, first, last)),
                    cost=0.34, av=True, vneed=kt))
                if (qc, pair) == (0, 1) and last:
                    avt01_t = sig_end[g] + 1.0
                    for qt in range(4):
                        box = {}
                        for nn in range(2):
                            queue.append(dict(
                                rel=avt01_t + qt * 0.4, dl=nslots - 8 + qt,
                                fn=(lambda qt=qt, nn=nn, box=box:
                                    op_nn(0, qt, nn, box, 'v')),
                                cost=0.5, gid=('o', qt, nn), opens=True,
                                closes=True, navt=True))
                if g + 1 < nslots:
                    bank_free = sig_end[g - 1] + L2 if g >= 1 else 0.0
                    target = max(sig_end[g] - 0.45, bank_free)
                    pump_until(target, g)
                    sps_cur = score(*slots[g + 1][:3])
                    vt['pe'] = max(vt['pe'], bank_free) + 0.42
                    sc_done = vt['pe']

            # ---- drain ----
            pump_until(1e9, nslots + 100)
            ost = ost_pool.tile([P, 2, 512], f16, tag="ost", name="os1_0")
            for nn in range(2):
                ops = mm_pool.tile([P, 512], f32, tag="mm",
                                   name=f"o1_0_{nn}")
                for pr in range(2):
                    nc.tensor.matmul(
                        ops[:], avt_sb[:, pr, 1, 0:P],
                        wvo_sb[:, 1,
                               pr * HID + nn * 512:pr * HID + (nn + 1) * 512],
                        start=(pr == 0), stop=(pr == 1))
                if nn == 0:
                    nc.vector.tensor_copy(ost[:, nn, :], ops[:])
                else:
                    nc.scalar.copy(ost[:, nn, :], ops[:])
            nc.sync.dma_start(out=po_ap[512:512 + P, :], in_=ost[:])
            # remaining out_proj(1) tiles: qt3 through mm_pool (freed by
            # op1 pr1 above) so its matmuls overlap qt1/qt2's evacuation
            box3 = {}
            op_nn(1, 3, 0, box3, 'v')
            op_nn(1, 3, 1, box3, 's')
            # qt1/qt2 in s_pool PSUM (free after the last sigmoid),
            # evac casts split across Vector/Scalar
            for qt in range(1, 3):
                osp = s_pool.tile([P, 2, 512], f32, tag="s",
                                  name=f"osp{qt}")
                for nn in range(2):
                    for pr in range(2):
                        nc.tensor.matmul(
                            osp[:, nn, :],
                            avt_sb[:, pr, 1, qt * P:(qt + 1) * P],
                            wvo_sb[:, 1, pr * HID + nn * 512:pr * HID + (nn + 1) * 512],
                            start=(pr == 0), stop=(pr == 1))
                ostq = ost_pool.tile([P, 2, 512], f16, tag="ost",
                                     name=f"os1_{qt}")
                nc.vector.tensor_copy(ostq[:, 0, :], osp[:, 0, :])
                nc.scalar.copy(ostq[:, 1, :], osp[:, 1, :])
                r0 = 512 + qt * P
                nc.sync.dma_start(out=po_ap[r0:r0 + P, :], in_=ostq[:])

    nc.compile()
    return nc


def _prep_in_maps(query, key, value, attn_mask, Wq, Wk, Wv, Wo):
    query = np.asarray(query, np.float32)
    key = np.asarray(key, np.float32)
    value = np.asarray(value, np.float32)
    mask = np.asarray(attn_mask)
    Wq = np.asarray(Wq, np.float32)
    Wk = np.asarray(Wk, np.float32)
    Wv = np.asarray(Wv, np.float32)
    Wo = np.asarray(Wo, np.float32)

    # Masked klen positions contribute exactly 0 (reference: sigmoid(-1e30)
    # == 0), so compact each batch to its unmasked positions, zero-padded
    # to a common multiple of 128.
    idxs = [np.nonzero(mask[b] != 0)[0] for b in range(BSZ)]
    klen_eff = max(len(ix) for ix in idxs)
    nkt = max(4, -(-klen_eff // P))
    klen_c = nkt * P

    nblk = (klen_c + 511) // 512
    klen_pad = nblk * 512

    def block_x(xT, width, pad_to):
        # [HID, width] -> [nblocks, 128, 8, 512] contiguous, zero-padded
        full = np.zeros((HID, pad_to), np.float16)
        full[:, :width] = xT
        nb = pad_to // 512
        return np.ascontiguousarray(
            full.reshape(HID // P, P, nb, 512).transpose(2, 1, 0, 3))

    kTc, vTc = [], []
    for b in range(BSZ):
        ix = idxs[b]
        kTc.append(block_x(key[b].T[:, ix].astype(np.float16), len(ix), klen_pad))
        vTc.append(block_x(value[b].T[:, ix].astype(np.float16), len(ix), klen_pad))

    qT0 = {}
    in_maps = []
    for core in range(N_CORES):
        b, g = divmod(core, 4)
        sl = slice(g * GSLICE, (g + 1) * GSLICE)
        if b not in qT0:
            qT0[b] = block_x(query[b].T.astype(np.float16), QLEN, QLEN)
        wq_h = (Wq[:, sl].astype(np.float16).reshape(HID // P, P, GSLICE)
                .transpose(1, 0, 2))
        wk_h = (Wk[:, sl].astype(np.float16).reshape(HID // P, P, GSLICE)
                .transpose(1, 0, 2))
        wv_h = (Wv[:, sl].astype(np.float16).reshape(HID // P, P, GSLICE)
                .transpose(1, 0, 2))
        wo_h = (Wo[sl, :].astype(np.float16).reshape(2, P, HID)
                .transpose(1, 0, 2))
        def wsplit(w_h, half):
            # [P, NC_, 256] -> [P, NC_, 128] for one head-pair half
            return w_h[:, :, half * P:(half + 1) * P]
        in_maps.append({
            "qT": qT0[b],
            "kT": kTc[b],
            "vT": vTc[b],
            "wkq": np.ascontiguousarray(
                np.stack([wsplit(wk_h, 0), wsplit(wq_h, 0),
                          wsplit(wq_h, 1), wsplit(wk_h, 1)])),
            "wvo": np.ascontiguousarray(
                np.stack([wv_h.reshape(P, 2 * HID),
                          wo_h.reshape(P, 2 * HID)])),
        })
    return in_maps, nkt


def _run(in_maps, nkt, trace):
    from concourse.bass_utils import run_bass_kernel_spmd

    if nkt not in _cache:
        _cache[nkt] = _build(nkt)
    res = run_bass_kernel_spmd(_cache[nkt], in_maps, list(range(N_CORES)),
                               trace=trace)
    out = np.zeros((BSZ, QLEN, HID), np.float32)
    for core in range(N_CORES):
        out[core // 4] += res.results[core]["po"].astype(np.float32)
    return out, res


def kernel(query, key, value, attn_mask, Wq, Wk, Wv, Wo):
    in_maps, nkt = _prep_in_maps(query, key, value, attn_mask, Wq, Wk, Wv, Wo)
    out, _ = _run(in_maps, nkt, trace=False)
    return out


def run_traced(query, key, value, attn_mask, Wq, Wk, Wv, Wo):
    """Like kernel() but with NTFF profiling; returns (out, exec_time_ns)."""
    in_maps, nkt = _prep_in_maps(query, key, value, attn_mask, Wq, Wk, Wv, Wo)
    out, res = _run(in_maps, nkt, trace=True)
    return out, res.exec_time_ns

